# revision 1
# baseline (speedup 1.0000x reference)
"""Trainium2 Bass kernel for nn_CrossAttention (B=4, T=1024, S=2048, D=1024, H=16).

Sharding: tensor-parallel over heads. Each of the 8 cores owns 2 heads
(a 128-column slice of the q/k/v projections and the matching 128-row
slice of the o-projection input). Every core computes a full-shape
partial o-proj output; the host sums the 8 partials (the all-reduce is
done on the host during the gather/unshard step).

Layout strategy: all device matmuls contract along the SBUF partition
axis, so the host pre-transposes the activations and weights (free on
host, removes every on-chip transpose):
  xT  [D, B*T]  = query^T          (bf16)
  kvT [D, B*S]  = key_value^T      (bf16)
  wqT/wkT/wvT [D, 128] per core    (bf16)
  woT [128, D] per core            (bf16)

Pipeline per core (all matmul accumulation in fp32 PSUM):
  qT = WqT.T @ xT            -> [128c, B*T]   (c on partitions)
  kT = WkT.T @ kvT           -> [128c, B*S]
  V  = kvT.T @ WvT           -> [s, c] tiles, stored ones-augmented [128, 65]
  scoresT = kT.T @ qT per head (K=64)         -> [128s, 512t] PSUM
  PT = exp(0.125 * scoresT)  (ACT, no max-subtraction: |scores| < ~7)
  attnT[h] += V_aug.T @ PT   -> [65, 512t]; row 64 = softmax rowsum (free)
  rowsum transposed to [t, 1] via K=1 matmuls, reciprocal on DVE
  o-proj per head (K=64, row-packed) -> [128t, dout] PSUM per head
  out = psum_h0 * (1/r0)[t] + psum_h1 * (1/r1)[t]   (per-partition scalars)
"""

import os
import sys
from contextlib import ExitStack

import numpy as np

for _p in (
    "/root/.axon_site",
    "/root/.axon_site/_ro/trn_rl_repo",
    "/root/.axon_site/_ro/pypackages",
    "/opt/trn_rl_repo",
):
    if os.path.isdir(_p) and _p not in sys.path:
        sys.path.append(_p)

import ml_dtypes  # noqa: E402

import concourse.bass as bass  # noqa: E402
import concourse.mybir as mybir  # noqa: E402
import concourse.tile as tile  # noqa: E402
from concourse import bacc  # noqa: E402

BF = mybir.dt.bfloat16
F32 = mybir.dt.float32
NPBF = ml_dtypes.bfloat16

B, T, S, D = 4, 1024, 2048, 1024
BT, BS = B * T, B * S
P = 128
NCORES = 8
KT = D // P          # 8 contraction tiles of 128
TCH = 512            # free-dim chunk for projections / attention t-chunks
NJ = S // P          # 16 s-tiles of 128 per batch
NST = BS // P        # 64 s-tiles total
EXP_SCALE = float(64 ** -0.5)  # folded into the ACT exp


def build_nc():
    nc = bacc.Bacc("TRN2", target_bir_lowering=False)

    xT = nc.dram_tensor("xT", [D, BT], BF, kind="ExternalInput").ap()
    kvT = nc.dram_tensor("kvT", [D, BS], BF, kind="ExternalInput").ap()
    wqT = nc.dram_tensor("wqT", [D, P], BF, kind="ExternalInput").ap()
    wkT = nc.dram_tensor("wkT", [D, P], BF, kind="ExternalInput").ap()
    wvT = nc.dram_tensor("wvT", [D, P], BF, kind="ExternalInput").ap()
    woT = nc.dram_tensor("woT", [P, D], BF, kind="ExternalInput").ap()
    eye2_d = nc.dram_tensor("eye2", [2, 2], F32, kind="ExternalInput").ap()
    out = nc.dram_tensor("out", [BT, D], F32, kind="ExternalOutput").ap()

    with tile.TileContext(nc) as tc, ExitStack() as ctx:
        consts = ctx.enter_context(tc.tile_pool(name="consts", bufs=1))
        big = ctx.enter_context(tc.tile_pool(name="big", bufs=1))
        xin = ctx.enter_context(tc.tile_pool(name="xin", bufs=2))
        ptp = ctx.enter_context(tc.tile_pool(name="ptp", bufs=4))
        atsb = ctx.enter_context(tc.tile_pool(name="atsb", bufs=2))
        smalls = ctx.enter_context(tc.tile_pool(name="smalls", bufs=4))
        outp = ctx.enter_context(tc.tile_pool(name="outp", bufs=3))
        # PSUM budget (8 banks): mm [128,1024]x2 = 4 + at 2 + op 2
        mm_ps = ctx.enter_context(tc.tile_pool(name="mm_ps", bufs=2, space="PSUM"))
        at_pool = ctx.enter_context(tc.tile_pool(name="at_ps", bufs=2, space="PSUM"))
        op_pool = ctx.enter_context(tc.tile_pool(name="op_ps", bufs=2, space="PSUM"))

        # ---- resident weights ----
        wq_s = consts.tile([P, KT, P], BF, tag="wq_s")
        wk_s = consts.tile([P, KT, P], BF, tag="wk_s")
        wv_s = consts.tile([P, KT, P], BF, tag="wv_s")
        wqT_t = wqT.rearrange("(kt p) c -> p kt c", p=P)
        wkT_t = wkT.rearrange("(kt p) c -> p kt c", p=P)
        wvT_t = wvT.rearrange("(kt p) c -> p kt c", p=P)
        for kt in range(KT):
            nc.sync.dma_start(wq_s[:, kt, :], wqT_t[:, kt, :])
            nc.sync.dma_start(wk_s[:, kt, :], wkT_t[:, kt, :])
            nc.sync.dma_start(wv_s[:, kt, :], wvT_t[:, kt, :])
        wo_s = consts.tile([P, D], BF, tag="wo_s")
        nc.sync.dma_start(wo_s[:], woT)
        # [1,1] ones: rhs for the K=1 rowsum-transpose matmuls
        ones1 = consts.tile([1, 1], F32, tag="ones1")
        nc.sync.dma_start(ones1[:], eye2_d[0:1, 0:1])

        # ---- resident intermediates ----
        qT_s = big.tile([P, BT], BF, tag="qT_s")
        kT_s = big.tile([P, BS], BF, tag="kT_s")
        # Per-head V, ones-augmented: 64 s-tiles, each [128, 65] with col 64 == 1.0
        v_s = [
            big.tile([P, NST * 65], BF, tag=f"v{h}_s", name=f"v{h}_s")
            for h in range(2)
        ]
        for h in range(2):
            nc.gpsimd.memset(v_s[h][:], 1.0)

        xT_t = xT.rearrange("(kt p) t -> p kt t", p=P)
        kvT_t = kvT.rearrange("(kt p) s -> p kt s", p=P)

        def load_x_batch(b):
            # whole batch: per-partition runs of 2KB -> good DMA descriptors
            x_t = xin.tile([P, KT, T], BF, tag="x_t", name="x_t")
            for kt in range(KT):
                nc.sync.dma_start(x_t[:, kt, :], xT_t[:, kt, b * T:(b + 1) * T])
            return x_t

        def load_kv_batch(b):
            kv_t = xin.tile([P, KT, S], BF, tag="kv_t", name="kv_t")
            for kt in range(KT):
                nc.sync.dma_start(kv_t[:, kt, :], kvT_t[:, kt, b * S:(b + 1) * S])
            return kv_t

        def q_proj_frags(x_t, b, half):
            # q projection for one 512-wide chunk, split into 2 fragments
            # (4 k-tiles each) so it can be spread across attention j-iters
            ch = 2 * b + half
            state = {}

            def frag(kts):
                def run():
                    if "ps" not in state:
                        state["ps"] = op_pool.tile([P, TCH], F32, tag="op", name="qps")
                    ps = state["ps"]
                    for kt in kts:
                        nc.tensor.matmul(
                            ps[:], wq_s[:, kt, :],
                            x_t[:, kt, half * TCH:(half + 1) * TCH],
                            start=(kt == 0), stop=(kt == KT - 1),
                        )
                    if KT - 1 in kts:
                        nc.vector.tensor_copy(
                            qT_s[:, ch * TCH:(ch + 1) * TCH], ps[:]
                        )
                return run

            return [frag(range(0, 4)), frag(range(4, KT))]

        def kv_proj_frags(kv_t, b, quarter):
            # kT projection (1 fragment) + V projection (2 fragments) for one
            # 512-wide kv chunk
            ch = 4 * b + quarter
            q0 = quarter * TCH
            state = {}

            def k_frag():
                ps = op_pool.tile([P, TCH], F32, tag="op", name="kps")
                for kt in range(KT):
                    nc.tensor.matmul(
                        ps[:], wk_s[:, kt, :], kv_t[:, kt, q0:q0 + TCH],
                        start=(kt == 0), stop=(kt == KT - 1),
                    )
                nc.vector.tensor_copy(kT_s[:, ch * TCH:(ch + 1) * TCH], ps[:])

            # V projection: [s, c] orientation, 4 s-subtiles share one bank.
            # start only on the bank's first matmul: start=True marks the
            # whole 2KB zero-region pending-zero, so later subtiles' first
            # writes overwrite (not accumulate) stale data automatically.
            def v_frag(kts):
                def run():
                    if "vps" not in state:
                        state["vps"] = op_pool.tile(
                            [P, 4, P], F32, tag="op", name="vps"
                        )
                    vps = state["vps"]
                    for kt in kts:
                        for sub in range(4):
                            nc.tensor.matmul(
                                vps[:, sub, :],
                                kv_t[:, kt, q0 + sub * P:q0 + (sub + 1) * P],
                                wv_s[:, kt, :],
                                start=(kt == 0 and sub == 0),
                                stop=(kt == KT - 1 and sub == 3),
                            )
                    if KT - 1 in kts:
                        for sub in range(4):
                            jg = ch * 4 + sub
                            nc.vector.tensor_copy(
                                v_s[0][:, jg * 65:jg * 65 + 64], vps[:, sub, 0:64]
                            )
                            nc.vector.tensor_copy(
                                v_s[1][:, jg * 65:jg * 65 + 64], vps[:, sub, 64:128]
                            )
                return run

            return [k_frag, v_frag(range(0, 4)), v_frag(range(4, KT))]

        def attention_block(b, t2, fillers=()):
            # fillers: [(j, fn)] — PE filler work (next batch's projections,
            # previous block's o-proj) issued after iteration j so the tensor
            # engine never idles long enough for the HAM clock gate to
            # re-throttle. Returns a list of (j, fn) tail closures (rowsum
            # transpose, o-proj, combine, store) to interleave into the NEXT
            # block.
            fmap = {}
            for j, fn in fillers:
                fmap.setdefault(j, []).append(fn)
            t0 = b * T + t2 * TCH
            ats = [
                at_pool.tile([65, TCH], F32, tag="at", name=f"at{h}")
                for h in range(2)
            ]
            for j in range(NJ):
                jg = b * NJ + j
                for fn in fmap.get(j, ()):
                    fn()
                sc = mm_ps.tile([P, 1024], F32, tag="mm", name="sc")
                for h in range(2):
                    hp = h * 64
                    nc.tensor.matmul(
                        sc[:, h * TCH:(h + 1) * TCH],
                        kT_s[hp:hp + 64, b * S + j * P: b * S + (j + 1) * P],
                        qT_s[hp:hp + 64, t0:t0 + TCH],
                        start=True, stop=True,
                    )
                pt = ptp.tile([P, 1024], BF, tag="pt", name="pt")
                nc.scalar.activation(
                    pt[:], sc[:],
                    mybir.ActivationFunctionType.Exp,
                    scale=EXP_SCALE,
                )
                for h in range(2):
                    nc.tensor.matmul(
                        ats[h][:],
                        v_s[h][:, jg * 65:(jg + 1) * 65],
                        pt[:, h * TCH:(h + 1) * TCH],
                        start=(j == 0), stop=(j == NJ - 1),
                    )

            # rowsums -> [t, 1] layout via K=1 matmuls against ones[1,1]:
            # rt_ps[:, h*4+sub] = r_h[t] for the sub'th 128-wide t range
            rt_ps = op_pool.tile([P, TCH], F32, tag="op", name="rt_ps")
            aT = atsb.tile([P, TCH], BF, tag="aT", name="aT")
            r_sb = smalls.tile([1, 2 * TCH], F32, tag="rsb", name="r_sb")
            for h in range(2):
                nc.vector.tensor_copy(
                    r_sb[0:1, h * TCH:(h + 1) * TCH], ats[h][64:65, :]
                )
                nc.vector.tensor_copy(aT[h * 64:(h + 1) * 64, :], ats[h][0:64, :])
            for seg in range(8):
                nc.tensor.matmul(
                    rt_ps[:, seg:seg + 1],
                    r_sb[0:1, seg * P:(seg + 1) * P],
                    ones1[0:1, 0:1],
                    start=True, stop=True,
                )
            rt = smalls.tile([P, 8], F32, tag="rtr", name="rt")
            nc.vector.reciprocal(rt[:], rt_ps[:, :8])

            # o-proj per head (K=64, row-packed pair); fused combine with 1/r
            for sub in range(4):
                ot = outp.tile([P, D], F32, tag="ot", name="ot")
                for n in range(D // TCH):  # 2
                    op0 = op_pool.tile([P, TCH], F32, tag="op", name="op0")
                    op1 = op_pool.tile([P, TCH], F32, tag="op", name="op1")
                    nc.tensor.matmul(
                        op0[:],
                        aT[0:64, sub * P:(sub + 1) * P],
                        wo_s[0:64, n * TCH:(n + 1) * TCH],
                        start=True, stop=True,
                    )
                    nc.tensor.matmul(
                        op1[:],
                        aT[64:128, sub * P:(sub + 1) * P],
                        wo_s[64:128, n * TCH:(n + 1) * TCH],
                        start=True, stop=True,
                    )
                    osl = ot[:, n * TCH:(n + 1) * TCH]
                    nc.vector.tensor_scalar_mul(osl, op1[:], rt[:, 4 + sub:5 + sub])
                    nc.vector.scalar_tensor_tensor(
                        osl, op0[:], rt[:, sub:sub + 1], osl,
                        mybir.AluOpType.mult, mybir.AluOpType.add,
                    )
                nc.sync.dma_start(out[t0 + sub * P:t0 + (sub + 1) * P, :], ot[:])

        # PE warmup: throwaway matmuls on the (tiny, fast-loading) weight
        # tiles engage the HAM clock gate to 8/8 during the DMA lead-in,
        # so the first real matmuls run at 2.4 GHz. Also pre-load the ACT
        # exp table so the first real exp doesn't pay the ~2.7us load.
        warm_sb = smalls.tile([1, 1], F32, tag="rsb", name="warm_sb")
        nc.scalar.activation(
            warm_sb[:], ones1[:], mybir.ActivationFunctionType.Exp
        )
        warm_ps = op_pool.tile([P, TCH], F32, tag="op", name="warm_ps")
        wq_flat = wq_s.rearrange("p k c -> p (k c)")
        for i in range(24):
            nc.tensor.matmul(
                warm_ps[:], wq_s[:, i % KT, :], wq_flat[:, :TCH],
                start=True, stop=True,
            )

        # b-major pipeline: batch 0 projects upfront; batch b+1's
        # projections are interleaved into batch b's attention as PE filler,
        # fragmented so no single j-iter gets a large PE burst (keeps the
        # ACT pipeline fed and the HAM clock gate warm).
        x_t = load_x_batch(0)
        kv_t = load_kv_batch(0)
        for fn in q_proj_frags(x_t, 0, 0) + q_proj_frags(x_t, 0, 1):
            fn()
        for quarter in range(4):
            for fn in kv_proj_frags(kv_t, 0, quarter):
                fn()
        for b in range(B):
            if b + 1 < B:
                nx_t = load_x_batch(b + 1)
                nkv_t = load_kv_batch(b + 1)
                frags0 = (
                    q_proj_frags(nx_t, b + 1, 0)
                    + q_proj_frags(nx_t, b + 1, 1)
                    + kv_proj_frags(nkv_t, b + 1, 0)
                )
                frags1 = (
                    kv_proj_frags(nkv_t, b + 1, 1)
                    + kv_proj_frags(nkv_t, b + 1, 2)
                    + kv_proj_frags(nkv_t, b + 1, 3)
                )
                f0 = list(enumerate(frags0, start=2))
                f1 = list(enumerate(frags1, start=2))
            else:
                f0, f1 = [], []
            attention_block(b, 0, f0)
            attention_block(b, 1, f1)

    nc.compile()
    return nc


_NC_CACHE = None


def _get_nc():
    global _NC_CACHE
    if _NC_CACHE is None:
        _NC_CACHE = build_nc()
    return _NC_CACHE


def make_in_maps(query, key_value, wq, wk, wv, wo):
    q2 = np.ascontiguousarray(np.asarray(query, np.float32).reshape(BT, D))
    kv2 = np.ascontiguousarray(np.asarray(key_value, np.float32).reshape(BS, D))
    xT = np.ascontiguousarray(q2.astype(NPBF).T)
    kvT = np.ascontiguousarray(kv2.astype(NPBF).T)
    wq = np.asarray(wq, np.float32)
    wk = np.asarray(wk, np.float32)
    wv = np.asarray(wv, np.float32)
    wo = np.asarray(wo, np.float32)
    in_maps = []
    for c in range(NCORES):
        cs = slice(c * P, (c + 1) * P)
        in_maps.append({
            "xT": xT,
            "kvT": kvT,
            "wqT": np.ascontiguousarray(wq[cs, :].astype(NPBF).T),
            "wkT": np.ascontiguousarray(wk[cs, :].astype(NPBF).T),
            "wvT": np.ascontiguousarray(wv[cs, :].astype(NPBF).T),
            "woT": np.ascontiguousarray(wo[:, cs].astype(NPBF).T),
            "eye2": np.eye(2, dtype=np.float32),
        })
    return in_maps


def run(inputs, trace=False, **kwargs):
    from concourse.bass_utils import run_bass_kernel_spmd

    nc = _get_nc()
    in_maps = make_in_maps(**inputs)
    res = run_bass_kernel_spmd(
        nc, in_maps, core_ids=list(range(NCORES)), trace=trace, **kwargs
    )
    acc = np.zeros((BT, D), np.float64)
    for r in res.results:
        acc += r["out"].astype(np.float64)
    return acc.astype(np.float32).reshape(B, T, D), res


def kernel(**inputs):
    return run(inputs, trace=False)[0]



# revision 7
# speedup vs baseline: 1.0504x; 1.0504x over previous
"""Trainium2 Bass kernel for nn_CrossAttention (B=4, T=1024, S=2048, D=1024, H=16).

Sharding: tensor-parallel over heads. Each of the 8 cores owns 2 heads
(a 128-column slice of the q/k/v projections and the matching 128-row
slice of the o-projection input). Every core computes a full-shape
partial o-proj output (bf16); the host sums the 8 partials (the
all-reduce is done on the host during the gather/unshard step).

Layout strategy: all device matmuls contract along the SBUF partition
axis, so the host pre-transposes the activations and weights (free on
host, removes every on-chip transpose):
  xT  [D, B*T]  = query^T          (bf16)
  kvT [D, B*S]  = key_value^T      (bf16)
  wqT/wkT/wvT [D, 128] per core    (bf16)
  woT [128, D] per core            (bf16)

Pipeline per core (all matmul accumulation in fp32 PSUM):
  qT = WqT.T @ xT            -> [128c, B*T]   (c on partitions)
  kT = WkT.T @ kvT           -> [128c, B*S]
  V  = kvT.T @ WvT           -> [s, c] tiles, stored ones-augmented [128, 65]
  scoresT = kT.T @ qT per head (K=64)         -> [128s, 512t] PSUM
  PT = exp(0.125 * scoresT)  (ACT, no max-subtraction: |scores| < ~7)
  attnT[h] += V_aug.T @ PT   -> [65, 512t]; row 64 = softmax rowsum (free)
  rinv = 1/rowsum (DVE) -> bf16 [1, 512]; rb[h] = ones64.T @ rinv
    (K=1 PE outer product broadcasts rinv across 64 partitions in PSUM)
  aT[h] = attnT[h] * rb[h]   (DVE scalar_tensor_tensor, bf16 out)
  o-proj: out[128t, d] = aT[:, tsub].T @ woT  (K=128, fp32 PSUM)
  out partial stored bf16; o-proj deferred into the next attention
  block's j-loop as tensor-engine filler (keeps the HAM clock gate at
  full speed).
"""

import os
import sys
from contextlib import ExitStack

import numpy as np

for _p in (
    "/root/.axon_site",
    "/root/.axon_site/_ro/trn_rl_repo",
    "/root/.axon_site/_ro/pypackages",
    "/opt/trn_rl_repo",
):
    if os.path.isdir(_p) and _p not in sys.path:
        sys.path.append(_p)

import ml_dtypes  # noqa: E402

import concourse.bass as bass  # noqa: E402
import concourse.mybir as mybir  # noqa: E402
import concourse.tile as tile  # noqa: E402
from concourse import bacc  # noqa: E402

BF = mybir.dt.bfloat16
F32 = mybir.dt.float32
NPBF = ml_dtypes.bfloat16

B, T, S, D = 4, 1024, 2048, 1024
BT, BS = B * T, B * S
P = 128
NCORES = 8
KT = D // P          # 8 contraction tiles of 128
TCH = 512            # free-dim chunk for projections / attention t-chunks
NJ = S // P          # 16 s-tiles of 128 per batch
NST = BS // P        # 64 s-tiles total
EXP_SCALE = float(64 ** -0.5)  # folded into the ACT exp


def build_nc():
    nc = bacc.Bacc("TRN2", target_bir_lowering=False)

    xT = nc.dram_tensor("xT", [D, BT], BF, kind="ExternalInput").ap()
    kvT = nc.dram_tensor("kvT", [D, BS], BF, kind="ExternalInput").ap()
    wqT = nc.dram_tensor("wqT", [D, P], BF, kind="ExternalInput").ap()
    wkT = nc.dram_tensor("wkT", [D, P], BF, kind="ExternalInput").ap()
    wvT = nc.dram_tensor("wvT", [D, P], BF, kind="ExternalInput").ap()
    woT = nc.dram_tensor("woT", [P, D], BF, kind="ExternalInput").ap()
    eye2_d = nc.dram_tensor("eye2", [2, 2], F32, kind="ExternalInput").ap()
    # sel2 = kron(eye(2), ones(64)): head-selector for the rowsum broadcast
    sel2_d = nc.dram_tensor("sel2", [2, P], BF, kind="ExternalInput").ap()
    out = nc.dram_tensor("out", [BT, D], BF, kind="ExternalOutput").ap()

    with tile.TileContext(nc) as tc, ExitStack() as ctx:
        consts = ctx.enter_context(tc.tile_pool(name="consts", bufs=1))
        big = ctx.enter_context(tc.tile_pool(name="big", bufs=1))
        xin = ctx.enter_context(tc.tile_pool(name="xin", bufs=2))
        ptp = ctx.enter_context(tc.tile_pool(name="ptp", bufs=4))
        atsb = ctx.enter_context(tc.tile_pool(name="atsb", bufs=6))
        smalls = ctx.enter_context(tc.tile_pool(name="smalls", bufs=4))
        outp = ctx.enter_context(tc.tile_pool(name="outp", bufs=3))
        # PSUM budget (8 banks): mm [128,1024]x2 = 4 + at 2 + op 2
        mm_ps = ctx.enter_context(tc.tile_pool(name="mm_ps", bufs=2, space="PSUM"))
        at_pool = ctx.enter_context(tc.tile_pool(name="at_ps", bufs=2, space="PSUM"))
        op_pool = ctx.enter_context(tc.tile_pool(name="op_ps", bufs=2, space="PSUM"))

        # ---- resident weights ----
        wq_s = consts.tile([P, KT, P], BF, tag="wq_s")
        wk_s = consts.tile([P, KT, P], BF, tag="wk_s")
        wv_s = consts.tile([P, KT, P], BF, tag="wv_s")
        wqT_t = wqT.rearrange("(kt p) c -> p kt c", p=P)
        wkT_t = wkT.rearrange("(kt p) c -> p kt c", p=P)
        wvT_t = wvT.rearrange("(kt p) c -> p kt c", p=P)
        for kt in range(KT):
            nc.sync.dma_start(wq_s[:, kt, :], wqT_t[:, kt, :])
            nc.sync.dma_start(wk_s[:, kt, :], wkT_t[:, kt, :])
            nc.sync.dma_start(wv_s[:, kt, :], wvT_t[:, kt, :])
        wo_s = consts.tile([P, D], BF, tag="wo_s")
        nc.sync.dma_start(wo_s[:], woT)
        # [1,1] ones (fp32, ACT warmup); [2,128] head selector (bf16)
        ones1 = consts.tile([1, 1], F32, tag="ones1")
        nc.sync.dma_start(ones1[:], eye2_d[0:1, 0:1])
        sel2 = consts.tile([2, P], BF, tag="sel2")
        nc.sync.dma_start(sel2[:], sel2_d)

        # ---- resident intermediates ----
        qT_s = big.tile([P, BT], BF, tag="qT_s")
        kT_s = big.tile([P, BS], BF, tag="kT_s")
        # Per-head V, ones-augmented: 64 s-tiles, each [128, 65] with col 64 == 1.0
        v_s = [
            big.tile([P, NST * 65], BF, tag=f"v{h}_s", name=f"v{h}_s")
            for h in range(2)
        ]
        for h in range(2):
            nc.gpsimd.memset(v_s[h][:], 1.0)

        xT_t = xT.rearrange("(kt p) t -> p kt t", p=P)
        kvT_t = kvT.rearrange("(kt p) s -> p kt s", p=P)

        def load_x_batch(b):
            # per-half loads so q-proj halves can start as data lands
            x_t = xin.tile([P, KT, T], BF, tag="x_t", name="x_t")
            for half in range(2):
                sl = slice(half * TCH, (half + 1) * TCH)
                for kt in range(KT):
                    nc.sync.dma_start(
                        x_t[:, kt, sl], xT_t[:, kt, b * T + half * TCH:
                                             b * T + (half + 1) * TCH]
                    )
            return x_t

        def load_kv_batch(b):
            # per-quarter loads so k/v-proj quarters start as data lands
            kv_t = xin.tile([P, KT, S], BF, tag="kv_t", name="kv_t")
            for quarter in range(4):
                sl = slice(quarter * TCH, (quarter + 1) * TCH)
                for kt in range(KT):
                    nc.sync.dma_start(
                        kv_t[:, kt, sl], kvT_t[:, kt, b * S + quarter * TCH:
                                               b * S + (quarter + 1) * TCH]
                    )
            return kv_t

        def q_proj_frags(x_t, b, half):
            # q projection for one 512-wide chunk, split into 2 fragments
            # (4 k-tiles each) so it can be spread across attention j-iters
            ch = 2 * b + half
            state = {}

            def frag(kts):
                def run():
                    if "ps" not in state:
                        state["ps"] = op_pool.tile([P, TCH], F32, tag="op", name="qps")
                    ps = state["ps"]
                    for kt in kts:
                        nc.tensor.matmul(
                            ps[:], wq_s[:, kt, :],
                            x_t[:, kt, half * TCH:(half + 1) * TCH],
                            start=(kt == 0), stop=(kt == KT - 1),
                        )
                    if KT - 1 in kts:
                        nc.vector.tensor_copy(
                            qT_s[:, ch * TCH:(ch + 1) * TCH], ps[:]
                        )
                return run

            return [frag(range(0, 4)), frag(range(4, KT))]

        def kv_proj_frags(kv_t, b, quarter):
            # kT projection (1 fragment) + V projection (2 fragments) for one
            # 512-wide kv chunk
            ch = 4 * b + quarter
            q0 = quarter * TCH
            state = {}

            def k_frag():
                ps = op_pool.tile([P, TCH], F32, tag="op", name="kps")
                for kt in range(KT):
                    nc.tensor.matmul(
                        ps[:], wk_s[:, kt, :], kv_t[:, kt, q0:q0 + TCH],
                        start=(kt == 0), stop=(kt == KT - 1),
                    )
                nc.vector.tensor_copy(kT_s[:, ch * TCH:(ch + 1) * TCH], ps[:])

            # V projection: [s, c] orientation, 4 s-subtiles share one bank.
            # start only on the bank's first matmul: start=True marks the
            # whole 2KB zero-region pending-zero, so later subtiles' first
            # writes overwrite (not accumulate) stale data automatically.
            def v_frag(kts):
                def run():
                    if "vps" not in state:
                        state["vps"] = op_pool.tile(
                            [P, 4, P], F32, tag="op", name="vps"
                        )
                    vps = state["vps"]
                    for kt in kts:
                        for sub in range(4):
                            nc.tensor.matmul(
                                vps[:, sub, :],
                                kv_t[:, kt, q0 + sub * P:q0 + (sub + 1) * P],
                                wv_s[:, kt, :],
                                start=(kt == 0 and sub == 0),
                                stop=(kt == KT - 1 and sub == 3),
                            )
                    if KT - 1 in kts:
                        for sub in range(4):
                            jg = ch * 4 + sub
                            nc.vector.tensor_copy(
                                v_s[0][:, jg * 65:jg * 65 + 64], vps[:, sub, 0:64]
                            )
                            nc.vector.tensor_copy(
                                v_s[1][:, jg * 65:jg * 65 + 64], vps[:, sub, 64:128]
                            )
                return run

            return [k_frag, v_frag(range(0, 4)), v_frag(range(4, KT))]

        def attention_block(b, t2, fillers=()):
            # fillers: [(j, fn)] — PE filler work (next batch's projections,
            # previous block's o-proj) issued after iteration j so the tensor
            # engine never idles long enough for the HAM clock gate to
            # re-throttle. Returns o-proj closures to interleave into the
            # NEXT block.
            fmap = {}
            for j, fn in fillers:
                fmap.setdefault(j, []).append(fn)
            t0 = b * T + t2 * TCH
            ats = [
                at_pool.tile([65, TCH], F32, tag="at", name=f"at{h}")
                for h in range(2)
            ]
            for j in range(NJ):
                jg = b * NJ + j
                for fn in fmap.get(j, ()):
                    fn()
                sc = mm_ps.tile([P, 1024], F32, tag="mm", name="sc")
                for h in range(2):
                    hp = h * 64
                    nc.tensor.matmul(
                        sc[:, h * TCH:(h + 1) * TCH],
                        kT_s[hp:hp + 64, b * S + j * P: b * S + (j + 1) * P],
                        qT_s[hp:hp + 64, t0:t0 + TCH],
                        start=True, stop=True,
                    )
                pt = ptp.tile([P, 1024], BF, tag="pt", name="pt")
                nc.scalar.activation(
                    pt[:], sc[:],
                    mybir.ActivationFunctionType.Exp,
                    scale=EXP_SCALE,
                )
                for h in range(2):
                    nc.tensor.matmul(
                        ats[h][:],
                        v_s[h][:, jg * 65:(jg + 1) * 65],
                        pt[:, h * TCH:(h + 1) * TCH],
                        start=(j == 0), stop=(j == NJ - 1),
                    )

            # --- normalization (inline: frees the at PSUM banks fast) ---
            # reciprocal rowsums from at row 64, free-major on partition 0
            # (DVE writes must start at a 32-aligned partition), then a tiny
            # SBUF->SBUF DMA repartitions to [2, TCH] for the matmul rhs.
            rinv = smalls.tile([1, 2, TCH], F32, tag="rinv", name="rinv")
            for h in range(2):
                nc.vector.reciprocal(rinv[0:1, h, :], ats[h][64:65, :])
            rinv_bf1 = smalls.tile([1, 2, TCH], BF, tag="rinvbf1", name="rinv_bf1")
            nc.vector.tensor_copy(rinv_bf1[:], rinv[:])
            rinv_bf = smalls.tile([2, TCH], BF, tag="rinvbf", name="rinv_bf")
            nc.sync.dma_start(rinv_bf[:], rinv_bf1[0:1, :, :])
            # unnormalized attention out -> SBUF bf16 (both heads packed)
            a_bf = atsb.tile([P, TCH], BF, tag="abf", name="a_bf")
            for h in range(2):
                nc.vector.tensor_copy(a_bf[h * 64:(h + 1) * 64, :], ats[h][0:64, :])
            # broadcast 1/r across partitions: rb = sel2.T @ rinv (K=2 matmul)
            # rows 0-63 = rinv[h0], rows 64-127 = rinv[h1]
            rb = at_pool.tile([P, TCH], F32, tag="at", name="rb")
            nc.tensor.matmul(rb[:], sel2[:], rinv_bf[:], start=True, stop=True)
            # aT = a_bf * rb  (normalized, bf16, o-proj stationary layout)
            aT = atsb.tile([P, TCH], BF, tag="aT", name="aT")
            nc.vector.scalar_tensor_tensor(
                aT[:], a_bf[:], 1.0, rb[:],
                mybir.AluOpType.bypass, mybir.AluOpType.mult,
            )

            # --- o-proj closures (deferred into the next block as filler) ---
            def oproj_sub(sub):
                def run():
                    ot = outp.tile([P, D], BF, tag="ot", name="ot")
                    for n in range(D // TCH):  # 2
                        ops = op_pool.tile([P, TCH], F32, tag="op", name="ops")
                        nc.tensor.matmul(
                            ops[:],
                            aT[:, sub * P:(sub + 1) * P],
                            wo_s[:, n * TCH:(n + 1) * TCH],
                            start=True, stop=True,
                        )
                        nc.vector.tensor_copy(ot[:, n * TCH:(n + 1) * TCH], ops[:])
                    nc.sync.dma_start(out[t0 + sub * P:t0 + (sub + 1) * P, :], ot[:])
                return run

            return [oproj_sub(sub) for sub in range(4)]

        # PE warmup: throwaway matmuls on the (tiny, fast-loading) weight
        # tiles engage the HAM clock gate to 8/8 during the DMA lead-in,
        # so the first real matmuls run at 2.4 GHz. Also pre-load the ACT
        # exp table so the first real exp doesn't pay the ~2.7us load.
        warm_sb = smalls.tile([1, 1], F32, tag="rinv", name="warm_sb")
        nc.scalar.activation(
            warm_sb[:], ones1[:], mybir.ActivationFunctionType.Exp
        )
        warm_ps = op_pool.tile([P, TCH], F32, tag="op", name="warm_ps")
        wq_flat = wq_s.rearrange("p k c -> p (k c)")
        for i in range(24):
            nc.tensor.matmul(
                warm_ps[:], wq_s[:, i % KT, :], wq_flat[:, :TCH],
                start=True, stop=True,
            )

        # b-major pipeline: batch 0 projects upfront; batch b+1's
        # projections are interleaved into batch b's attention as PE filler,
        # fragmented so no single j-iter gets a large PE burst (keeps the
        # ACT pipeline fed and the HAM clock gate warm). Each block also
        # carries the previous block's o-proj as filler.
        x_t = load_x_batch(0)
        kv_t = load_kv_batch(0)
        for fn in q_proj_frags(x_t, 0, 0) + q_proj_frags(x_t, 0, 1):
            fn()
        for quarter in range(4):
            for fn in kv_proj_frags(kv_t, 0, quarter):
                fn()
        otail = []
        for b in range(B):
            if b + 1 < B:
                nx_t = load_x_batch(b + 1)
                nkv_t = load_kv_batch(b + 1)
                frags0 = (
                    q_proj_frags(nx_t, b + 1, 0)
                    + q_proj_frags(nx_t, b + 1, 1)
                    + kv_proj_frags(nkv_t, b + 1, 0)
                )
                frags1 = (
                    kv_proj_frags(nkv_t, b + 1, 1)
                    + kv_proj_frags(nkv_t, b + 1, 2)
                    + kv_proj_frags(nkv_t, b + 1, 3)
                )
            else:
                frags0, frags1 = [], []
            f0 = list(enumerate(otail + frags0, start=1))
            otail = attention_block(b, 0, f0)
            f1 = list(enumerate(otail + frags1, start=1))
            otail = attention_block(b, 1, f1)
        for fn in otail:
            fn()

    nc.compile()
    return nc


_NC_CACHE = None


def _get_nc():
    global _NC_CACHE
    if _NC_CACHE is None:
        _NC_CACHE = build_nc()
    return _NC_CACHE


def make_in_maps(query, key_value, wq, wk, wv, wo):
    q2 = np.ascontiguousarray(np.asarray(query, np.float32).reshape(BT, D))
    kv2 = np.ascontiguousarray(np.asarray(key_value, np.float32).reshape(BS, D))
    xT = np.ascontiguousarray(q2.astype(NPBF).T)
    kvT = np.ascontiguousarray(kv2.astype(NPBF).T)
    wq = np.asarray(wq, np.float32)
    wk = np.asarray(wk, np.float32)
    wv = np.asarray(wv, np.float32)
    wo = np.asarray(wo, np.float32)
    in_maps = []
    for c in range(NCORES):
        cs = slice(c * P, (c + 1) * P)
        in_maps.append({
            "xT": xT,
            "kvT": kvT,
            "wqT": np.ascontiguousarray(wq[cs, :].astype(NPBF).T),
            "wkT": np.ascontiguousarray(wk[cs, :].astype(NPBF).T),
            "wvT": np.ascontiguousarray(wv[cs, :].astype(NPBF).T),
            "woT": np.ascontiguousarray(wo[:, cs].astype(NPBF).T),
            "eye2": np.eye(2, dtype=np.float32),
            "sel2": np.kron(np.eye(2), np.ones((1, 64))).astype(NPBF),
        })
    return in_maps


def run(inputs, trace=False, **kwargs):
    from concourse.bass_utils import run_bass_kernel_spmd

    nc = _get_nc()
    in_maps = make_in_maps(**inputs)
    res = run_bass_kernel_spmd(
        nc, in_maps, core_ids=list(range(NCORES)), trace=trace, **kwargs
    )
    acc = np.zeros((BT, D), np.float64)
    for r in res.results:
        acc += r["out"].astype(np.float64)
    return acc.astype(np.float32).reshape(B, T, D), res


def kernel(**inputs):
    return run(inputs, trace=False)[0]


# revision 15
# speedup vs baseline: 1.2537x; 1.1935x over previous
"""Trainium2 Bass kernel for nn_CrossAttention (B=4, T=1024, S=2048, D=1024, H=16).

Sharding: tensor-parallel over heads. Each of the 8 cores owns 2 heads
(a 128-column slice of the q/k/v projections and the matching 128-row
slice of the o-projection input). Every core computes a full-shape
partial o-proj output (bf16); the host sums the 8 partials (the
all-reduce is done on the host during the gather/unshard step).

Layout strategy: all device matmuls contract along the SBUF partition
axis, so the host pre-transposes the activations and weights (free on
host, removes every on-chip transpose):
  xT  [D, B*T]  = query^T          (bf16)
  kvT [D, B*S]  = key_value^T      (bf16)
  wqT/wkT/wvT [D, 128] per core    (bf16)
  woT [128, D] per core            (bf16)

Pipeline per core (all matmul accumulation in fp32 PSUM):
  qT = WqT.T @ xT            -> [128c, B*T]   (c on partitions)
  kT = WkT.T @ kvT           -> [128c, B*S]
  V  = kvT.T @ WvT           -> [s, c] tiles, stored ones-augmented [128, 65]
  scoresT = kT.T @ qT per head (K=64)         -> [128s, 512t] PSUM
  PT = exp(0.125 * scoresT)  (ACT, no max-subtraction: |scores| < ~7)
  attnT[h] += V_aug.T @ PT   -> [65, 512t]; row 64 = softmax rowsum (free)
  rinv = 1/rowsum (DVE) -> bf16 [1, 512]; rb[h] = ones64.T @ rinv
    (K=1 PE outer product broadcasts rinv across 64 partitions in PSUM)
  aT[h] = attnT[h] * rb[h]   (DVE scalar_tensor_tensor, bf16 out)
  o-proj: out[128t, d] = aT[:, tsub].T @ woT  (K=128, fp32 PSUM)
  out partial stored bf16; o-proj deferred into the next attention
  block's j-loop as tensor-engine filler (keeps the HAM clock gate at
  full speed).
"""

import os
import sys
from contextlib import ExitStack

import numpy as np

for _p in (
    "/root/.axon_site",
    "/root/.axon_site/_ro/trn_rl_repo",
    "/root/.axon_site/_ro/pypackages",
    "/opt/trn_rl_repo",
):
    if os.path.isdir(_p) and _p not in sys.path:
        sys.path.append(_p)

import ml_dtypes  # noqa: E402

import concourse.bass as bass  # noqa: E402
import concourse.mybir as mybir  # noqa: E402
import concourse.tile as tile  # noqa: E402
from concourse import bacc  # noqa: E402

BF = mybir.dt.bfloat16
F32 = mybir.dt.float32
NPBF = ml_dtypes.bfloat16

B, T, S, D = 4, 1024, 2048, 1024
BT, BS = B * T, B * S
P = 128
NCORES = 8
KT = D // P          # 8 contraction tiles of 128
TCH = 512            # free-dim chunk for projections / attention t-chunks
NJ = S // P          # 16 s-tiles of 128 per batch
NST = BS // P        # 64 s-tiles total
EXP_SCALE = float(64 ** -0.5)  # folded into the ACT exp


def build_nc():
    nc = bacc.Bacc("TRN2", target_bir_lowering=False)

    xT = nc.dram_tensor("xT", [D, BT], BF, kind="ExternalInput").ap()
    kvT = nc.dram_tensor("kvT", [D, BS], BF, kind="ExternalInput").ap()
    wqT = nc.dram_tensor("wqT", [D, P], BF, kind="ExternalInput").ap()
    wkT = nc.dram_tensor("wkT", [D, P], BF, kind="ExternalInput").ap()
    wvT = nc.dram_tensor("wvT", [D, P], BF, kind="ExternalInput").ap()
    woT = nc.dram_tensor("woT", [P, D], BF, kind="ExternalInput").ap()
    eye2_d = nc.dram_tensor("eye2", [2, 2], F32, kind="ExternalInput").ap()
    out = nc.dram_tensor("out", [BT, D], BF, kind="ExternalOutput").ap()

    with tile.TileContext(nc) as tc, ExitStack() as ctx:
        consts = ctx.enter_context(tc.tile_pool(name="consts", bufs=1))
        big = ctx.enter_context(tc.tile_pool(name="big", bufs=1))
        xin = ctx.enter_context(tc.tile_pool(name="xin", bufs=2))
        ptp = ctx.enter_context(tc.tile_pool(name="ptp", bufs=4))
        atsb = ctx.enter_context(tc.tile_pool(name="atsb", bufs=6))
        smalls = ctx.enter_context(tc.tile_pool(name="smalls", bufs=2))
        outp = ctx.enter_context(tc.tile_pool(name="outp", bufs=3))
        # PSUM budget (8 banks): mm [128,1024]x2 = 4 + at 2 + op 2
        mm_ps = ctx.enter_context(tc.tile_pool(name="mm_ps", bufs=2, space="PSUM"))
        at_pool = ctx.enter_context(tc.tile_pool(name="at_ps", bufs=2, space="PSUM"))
        op_pool = ctx.enter_context(tc.tile_pool(name="op_ps", bufs=2, space="PSUM"))

        # ---- resident weights ----
        wq_s = consts.tile([P, KT, P], BF, tag="wq_s")
        wk_s = consts.tile([P, KT, P], BF, tag="wk_s")
        wv_s = consts.tile([P, KT, P], BF, tag="wv_s")
        wqT_t = wqT.rearrange("(kt p) c -> p kt c", p=P)
        wkT_t = wkT.rearrange("(kt p) c -> p kt c", p=P)
        wvT_t = wvT.rearrange("(kt p) c -> p kt c", p=P)
        for kt in range(KT):
            nc.sync.dma_start(wq_s[:, kt, :], wqT_t[:, kt, :])
            nc.sync.dma_start(wk_s[:, kt, :], wkT_t[:, kt, :])
            nc.sync.dma_start(wv_s[:, kt, :], wvT_t[:, kt, :])
        wo_s = consts.tile([P, D], BF, tag="wo_s")
        nc.sync.dma_start(wo_s[:], woT)
        # [1,1] ones (fp32, ACT warmup)
        ones1 = consts.tile([1, 1], F32, tag="ones1")
        nc.sync.dma_start(ones1[:], eye2_d[0:1, 0:1])

        # ---- resident intermediates ----
        qT_s = big.tile([P, BT], BF, tag="qT_s")
        kT_s = big.tile([P, BS], BF, tag="kT_s")
        # Per-head V, ones-augmented: 64 s-tiles, each [128, 65] with col 64 == 1.0
        v_s = [
            big.tile([P, NST * 65], BF, tag=f"v{h}_s", name=f"v{h}_s")
            for h in range(2)
        ]
        for h in range(2):
            nc.gpsimd.memset(v_s[h][:], 1.0)

        xT_t = xT.rearrange("(kt p) t -> p kt t", p=P)
        kvT_t = kvT.rearrange("(kt p) s -> p kt s", p=P)

        def load_x_batch(b, split=1):
            # split>1: finer-grained loads so b0's q-proj starts as data lands
            x_t = xin.tile([P, KT, T], BF, tag="x_t", name="x_t")
            csz = T // split
            for c in range(split):
                sl = slice(c * csz, (c + 1) * csz)
                for kt in range(KT):
                    nc.sync.dma_start(
                        x_t[:, kt, sl],
                        xT_t[:, kt, b * T + c * csz: b * T + (c + 1) * csz],
                    )
            return x_t

        def load_kv_batch(b, split=1):
            kv_t = xin.tile([P, KT, S], BF, tag="kv_t", name="kv_t")
            csz = S // split
            for c in range(split):
                sl = slice(c * csz, (c + 1) * csz)
                for kt in range(KT):
                    nc.sync.dma_start(
                        kv_t[:, kt, sl],
                        kvT_t[:, kt, b * S + c * csz: b * S + (c + 1) * csz],
                    )
            return kv_t

        def q_proj_frags(x_t, b, half):
            # q projection for one 512-wide chunk, split into 2 fragments
            # (4 k-tiles each) so it can be spread across attention j-iters
            ch = 2 * b + half
            state = {}

            def frag(kts):
                def run():
                    if "ps" not in state:
                        state["ps"] = op_pool.tile([P, TCH], F32, tag="op", name="qps")
                    ps = state["ps"]
                    for kt in kts:
                        nc.tensor.matmul(
                            ps[:], wq_s[:, kt, :],
                            x_t[:, kt, half * TCH:(half + 1) * TCH],
                            start=(kt == 0), stop=(kt == KT - 1),
                        )
                    if KT - 1 in kts:
                        nc.vector.tensor_copy(
                            qT_s[:, ch * TCH:(ch + 1) * TCH], ps[:]
                        )
                return run

            return [frag(range(0, 4)), frag(range(4, KT))]

        def kv_proj_frags(kv_t, b, quarter):
            # kT projection (1 fragment) + V projection (2 fragments) for one
            # 512-wide kv chunk
            ch = 4 * b + quarter
            q0 = quarter * TCH
            state = {}

            def k_frag():
                ps = op_pool.tile([P, TCH], F32, tag="op", name="kps")
                for kt in range(KT):
                    nc.tensor.matmul(
                        ps[:], wk_s[:, kt, :], kv_t[:, kt, q0:q0 + TCH],
                        start=(kt == 0), stop=(kt == KT - 1),
                    )
                nc.vector.tensor_copy(kT_s[:, ch * TCH:(ch + 1) * TCH], ps[:])

            # V projection: [s, c] orientation, 4 s-subtiles share one bank.
            # start only on the bank's first matmul: start=True marks the
            # whole 2KB zero-region pending-zero, so later subtiles' first
            # writes overwrite (not accumulate) stale data automatically.
            def v_frag(kts):
                def run():
                    if "vps" not in state:
                        state["vps"] = op_pool.tile(
                            [P, 4, P], F32, tag="op", name="vps"
                        )
                    vps = state["vps"]
                    for kt in kts:
                        for sub in range(4):
                            nc.tensor.matmul(
                                vps[:, sub, :],
                                kv_t[:, kt, q0 + sub * P:q0 + (sub + 1) * P],
                                wv_s[:, kt, :],
                                start=(kt == 0 and sub == 0),
                                stop=(kt == KT - 1 and sub == 3),
                            )
                    if KT - 1 in kts:
                        for sub in range(4):
                            jg = ch * 4 + sub
                            nc.vector.tensor_copy(
                                v_s[0][:, jg * 65:jg * 65 + 64], vps[:, sub, 0:64]
                            )
                            nc.vector.tensor_copy(
                                v_s[1][:, jg * 65:jg * 65 + 64], vps[:, sub, 64:128]
                            )
                return run

            return [k_frag, v_frag(range(0, 4)), v_frag(range(4, KT))]

        def attention_block(b, t2, fillers=()):
            # fillers: [(j, fn)] — PE filler work (next batch's projections,
            # previous block's o-proj) issued after iteration j so the tensor
            # engine never idles long enough for the HAM clock gate to
            # re-throttle. Returns o-proj closures to interleave into the
            # NEXT block.
            fmap = {}
            for j, fn in fillers:
                fmap.setdefault(j, []).append(fn)
            t0 = b * T + t2 * TCH
            ats = [
                at_pool.tile([65, TCH], F32, tag="at", name=f"at{h}")
                for h in range(2)
            ]
            for j in range(NJ):
                jg = b * NJ + j
                for fn in fmap.get(j, ()):
                    fn()
                sc = mm_ps.tile([P, 1024], F32, tag="mm", name="sc")
                for h in range(2):
                    hp = h * 64
                    nc.tensor.matmul(
                        sc[:, h * TCH:(h + 1) * TCH],
                        kT_s[hp:hp + 64, b * S + j * P: b * S + (j + 1) * P],
                        qT_s[hp:hp + 64, t0:t0 + TCH],
                        start=True, stop=True,
                    )
                pt = ptp.tile([P, 1024], BF, tag="pt", name="pt")
                nc.scalar.activation(
                    pt[:], sc[:],
                    mybir.ActivationFunctionType.Exp,
                    scale=EXP_SCALE,
                )
                for h in range(2):
                    nc.tensor.matmul(
                        ats[h][:],
                        v_s[h][:, jg * 65:(jg + 1) * 65],
                        pt[:, h * TCH:(h + 1) * TCH],
                        start=(j == 0), stop=(j == NJ - 1),
                    )

            # --- normalization (inline: frees the at PSUM banks fast) ---
            # rowsums -> SBUF partition 0 first (custom-DVE ops misread
            # PSUM/base-64 sources on HW; plain casts handle them fine)
            rsum = smalls.tile([1, 2, TCH], F32, tag="rsum", name="rsum")
            for h in range(2):
                nc.vector.tensor_copy(rsum[0:1, h, :], ats[h][64:65, :])
            # unnormalized attention out -> SBUF bf16 (both heads packed)
            a_bf = atsb.tile([P, TCH], BF, tag="abf", name="a_bf")
            for h in range(2):
                nc.vector.tensor_copy(a_bf[h * 64:(h + 1) * 64, :], ats[h][0:64, :])
            # approx-reciprocal (51 ULP, plenty for softmax; rowsums are in
            # [~2, ~2e6] so no edge cases)
            rinv = smalls.tile([1, 2, TCH], F32, tag="rinv", name="rinv")
            nc.vector.reciprocal_approx_fast(rinv[0:1, :, :], rsum[0:1, :, :])
            rinv_bf = smalls.tile([1, 2, TCH], BF, tag="rinvbf", name="rinv_bf")
            nc.vector.tensor_copy(rinv_bf[:], rinv[:])
            # broadcast rinv across all partitions on the (idle) gpsimd
            # engine; row p of rbs = [rinv_h0 | rinv_h1] (1KB bf16/partition)
            rbs = atsb.tile([P, 2, TCH], BF, tag="rbs", name="rbs")
            nc.gpsimd.partition_broadcast(rbs[:], rinv_bf[0:1, :, :])
            # aT = a_bf * rinv  (normalized, bf16, o-proj stationary layout)
            aT = atsb.tile([P, TCH], BF, tag="aT", name="aT")
            for h in range(2):
                nc.vector.scalar_tensor_tensor(
                    aT[h * 64:(h + 1) * 64, :],
                    a_bf[h * 64:(h + 1) * 64, :], 1.0,
                    rbs[h * 64:(h + 1) * 64, h, :],
                    mybir.AluOpType.bypass, mybir.AluOpType.mult,
                )

            # --- o-proj closures (deferred into the next block as filler) ---
            def oproj_sub(sub):
                def run():
                    ot = outp.tile([P, D], BF, tag="ot", name="ot")
                    for n in range(D // TCH):  # 2
                        ops = op_pool.tile([P, TCH], F32, tag="op", name="ops")
                        nc.tensor.matmul(
                            ops[:],
                            aT[:, sub * P:(sub + 1) * P],
                            wo_s[:, n * TCH:(n + 1) * TCH],
                            start=True, stop=True,
                        )
                        nc.vector.tensor_copy(ot[:, n * TCH:(n + 1) * TCH], ops[:])
                    nc.sync.dma_start(out[t0 + sub * P:t0 + (sub + 1) * P, :], ot[:])
                return run

            return [oproj_sub(sub) for sub in range(4)]

        # PE warmup: throwaway matmuls on the (tiny, fast-loading) weight
        # tiles engage the HAM clock gate to 8/8 during the DMA lead-in,
        # so the first real matmuls run at 2.4 GHz. Also pre-load the ACT
        # exp table so the first real exp doesn't pay the ~2.7us load.
        warm_sb = smalls.tile([1, 1], F32, tag="rinv", name="warm_sb")
        nc.scalar.activation(
            warm_sb[:], ones1[:], mybir.ActivationFunctionType.Exp
        )
        warm_ps = op_pool.tile([P, TCH], F32, tag="op", name="warm_ps")
        wq_flat = wq_s.rearrange("p k c -> p (k c)")
        for i in range(24):
            nc.tensor.matmul(
                warm_ps[:], wq_s[:, i % KT, :], wq_flat[:, :TCH],
                start=True, stop=True,
            )

        # b-major pipeline: batch 0 projects upfront; batch b+1's
        # projections are interleaved into batch b's attention as PE filler,
        # fragmented so no single j-iter gets a large PE burst (keeps the
        # ACT pipeline fed and the HAM clock gate warm). Each block also
        # carries the previous block's o-proj as filler.
        x_t = load_x_batch(0, split=2)
        kv_t = load_kv_batch(0, split=4)
        for fn in q_proj_frags(x_t, 0, 0) + q_proj_frags(x_t, 0, 1):
            fn()
        for quarter in range(4):
            for fn in kv_proj_frags(kv_t, 0, quarter):
                fn()
        otail = []
        for b in range(B):
            if b + 1 < B:
                nx_t = load_x_batch(b + 1)
                nkv_t = load_kv_batch(b + 1)
                frags0 = (
                    q_proj_frags(nx_t, b + 1, 0)
                    + q_proj_frags(nx_t, b + 1, 1)
                    + kv_proj_frags(nkv_t, b + 1, 0)
                )
                frags1 = (
                    kv_proj_frags(nkv_t, b + 1, 1)
                    + kv_proj_frags(nkv_t, b + 1, 2)
                    + kv_proj_frags(nkv_t, b + 1, 3)
                )
            else:
                frags0, frags1 = [], []
            f0 = list(enumerate(otail + frags0, start=1))
            otail = attention_block(b, 0, f0)
            f1 = list(enumerate(otail + frags1, start=1))
            otail = attention_block(b, 1, f1)
        for fn in otail:
            fn()

    nc.compile()
    return nc


_NC_CACHE = None


def _get_nc():
    global _NC_CACHE
    if _NC_CACHE is None:
        _NC_CACHE = build_nc()
    return _NC_CACHE


def make_in_maps(query, key_value, wq, wk, wv, wo):
    q2 = np.ascontiguousarray(np.asarray(query, np.float32).reshape(BT, D))
    kv2 = np.ascontiguousarray(np.asarray(key_value, np.float32).reshape(BS, D))
    xT = np.ascontiguousarray(q2.astype(NPBF).T)
    kvT = np.ascontiguousarray(kv2.astype(NPBF).T)
    wq = np.asarray(wq, np.float32)
    wk = np.asarray(wk, np.float32)
    wv = np.asarray(wv, np.float32)
    wo = np.asarray(wo, np.float32)
    in_maps = []
    for c in range(NCORES):
        cs = slice(c * P, (c + 1) * P)
        in_maps.append({
            "xT": xT,
            "kvT": kvT,
            "wqT": np.ascontiguousarray(wq[cs, :].astype(NPBF).T),
            "wkT": np.ascontiguousarray(wk[cs, :].astype(NPBF).T),
            "wvT": np.ascontiguousarray(wv[cs, :].astype(NPBF).T),
            "woT": np.ascontiguousarray(wo[:, cs].astype(NPBF).T),
            "eye2": np.eye(2, dtype=np.float32),
        })
    return in_maps


def run(inputs, trace=False, **kwargs):
    from concourse.bass_utils import run_bass_kernel_spmd

    nc = _get_nc()
    in_maps = make_in_maps(**inputs)
    res = run_bass_kernel_spmd(
        nc, in_maps, core_ids=list(range(NCORES)), trace=trace, **kwargs
    )
    acc = np.zeros((BT, D), np.float64)
    for r in res.results:
        acc += r["out"].astype(np.float64)
    return acc.astype(np.float32).reshape(B, T, D), res


def kernel(**inputs):
    return run(inputs, trace=False)[0]


# revision 17
# speedup vs baseline: 1.3399x; 1.0687x over previous
"""Trainium2 Bass kernel for nn_CrossAttention (B=4, T=1024, S=2048, D=1024, H=16).

Sharding: tensor-parallel over heads. Each of the 8 cores owns 2 heads
(a 128-column slice of the q/k/v projections and the matching 128-row
slice of the o-projection input). Every core computes a full-shape
partial o-proj output (bf16); the host sums the 8 partials (the
all-reduce is done on the host during the gather/unshard step).

Layout strategy: all device matmuls contract along the SBUF partition
axis, so the host pre-transposes the activations and weights (free on
host, removes every on-chip transpose):
  xT  [D, B*T]  = query^T          (bf16)
  kvT [D, B*S]  = key_value^T      (bf16)
  wqT/wkT/wvT [D, 128] per core    (bf16)
  woT [128, D] per core            (bf16)

Pipeline per core (all matmul accumulation in fp32 PSUM):
  qT = WqT.T @ xT            -> [128c, B*T]   (c on partitions)
  kT = WkT.T @ kvT           -> [128c, B*S]
  V  = kvT.T @ WvT           -> [s, c] tiles, stored ones-augmented [128, 65]
  scoresT = kT.T @ qT per head (K=64)         -> [128s, 512t] PSUM
  PT = exp(0.125 * scoresT)  (ACT, no max-subtraction: |scores| < ~7)
  attnT[h] += V_aug.T @ PT   -> [65, 512t]; row 64 = softmax rowsum (free)
  rinv = 1/rowsum (DVE) -> bf16 [1, 512]; rb[h] = ones64.T @ rinv
    (K=1 PE outer product broadcasts rinv across 64 partitions in PSUM)
  aT[h] = attnT[h] * rb[h]   (DVE scalar_tensor_tensor, bf16 out)
  o-proj: out[128t, d] = aT[:, tsub].T @ woT  (K=128, fp32 PSUM)
  out partial stored bf16; o-proj deferred into the next attention
  block's j-loop as tensor-engine filler (keeps the HAM clock gate at
  full speed).
"""

import os
import sys
from contextlib import ExitStack

import numpy as np

for _p in (
    "/root/.axon_site",
    "/root/.axon_site/_ro/trn_rl_repo",
    "/root/.axon_site/_ro/pypackages",
    "/opt/trn_rl_repo",
):
    if os.path.isdir(_p) and _p not in sys.path:
        sys.path.append(_p)

import ml_dtypes  # noqa: E402

import concourse.bass as bass  # noqa: E402
import concourse.mybir as mybir  # noqa: E402
import concourse.tile as tile  # noqa: E402
from concourse import bacc  # noqa: E402

BF = mybir.dt.bfloat16
F32 = mybir.dt.float32
NPBF = ml_dtypes.bfloat16

B, T, S, D = 4, 1024, 2048, 1024
BT, BS = B * T, B * S
P = 128
NCORES = 8
KT = D // P          # 8 contraction tiles of 128
TCH = 512            # free-dim chunk for projections / attention t-chunks
NJ = S // P          # 16 s-tiles of 128 per batch
NST = BS // P        # 64 s-tiles total
EXP_SCALE = float(64 ** -0.5)  # folded into the ACT exp


def build_nc():
    nc = bacc.Bacc("TRN2", target_bir_lowering=False)

    xT = nc.dram_tensor("xT", [D, BT], BF, kind="ExternalInput").ap()
    kvT = nc.dram_tensor("kvT", [D, BS], BF, kind="ExternalInput").ap()
    wqT = nc.dram_tensor("wqT", [D, P], BF, kind="ExternalInput").ap()
    wkT = nc.dram_tensor("wkT", [D, P], BF, kind="ExternalInput").ap()
    wvT = nc.dram_tensor("wvT", [D, P], BF, kind="ExternalInput").ap()
    woT = nc.dram_tensor("woT", [P, D], BF, kind="ExternalInput").ap()
    eye2_d = nc.dram_tensor("eye2", [2, 2], F32, kind="ExternalInput").ap()
    out = nc.dram_tensor("out", [BT, D], BF, kind="ExternalOutput").ap()

    with tile.TileContext(nc) as tc, ExitStack() as ctx:
        consts = ctx.enter_context(tc.tile_pool(name="consts", bufs=1))
        big = ctx.enter_context(tc.tile_pool(name="big", bufs=1))
        xin = ctx.enter_context(tc.tile_pool(name="xin", bufs=2))
        ptp = ctx.enter_context(tc.tile_pool(name="ptp", bufs=4))
        atsb = ctx.enter_context(tc.tile_pool(name="atsb", bufs=6))
        smalls = ctx.enter_context(tc.tile_pool(name="smalls", bufs=2))
        outp = ctx.enter_context(tc.tile_pool(name="outp", bufs=3))
        # PSUM budget (8 banks): mm [128,1024]x2 = 4 + at 2 + op 2
        mm_ps = ctx.enter_context(tc.tile_pool(name="mm_ps", bufs=2, space="PSUM"))
        at_pool = ctx.enter_context(tc.tile_pool(name="at_ps", bufs=2, space="PSUM"))
        op_pool = ctx.enter_context(tc.tile_pool(name="op_ps", bufs=2, space="PSUM"))

        # ---- resident weights ----
        wq_s = consts.tile([P, KT, P], BF, tag="wq_s")
        wk_s = consts.tile([P, KT, P], BF, tag="wk_s")
        wv_s = consts.tile([P, KT, P], BF, tag="wv_s")
        wqT_t = wqT.rearrange("(kt p) c -> p kt c", p=P)
        wkT_t = wkT.rearrange("(kt p) c -> p kt c", p=P)
        wvT_t = wvT.rearrange("(kt p) c -> p kt c", p=P)
        for kt in range(KT):
            nc.sync.dma_start(wq_s[:, kt, :], wqT_t[:, kt, :])
            nc.sync.dma_start(wk_s[:, kt, :], wkT_t[:, kt, :])
            nc.sync.dma_start(wv_s[:, kt, :], wvT_t[:, kt, :])
        wo_s = consts.tile([P, D], BF, tag="wo_s")
        nc.sync.dma_start(wo_s[:], woT)
        # [1,1] ones (fp32, ACT warmup)
        ones1 = consts.tile([1, 1], F32, tag="ones1")
        nc.sync.dma_start(ones1[:], eye2_d[0:1, 0:1])

        # ---- resident intermediates ----
        qT_s = big.tile([P, BT], BF, tag="qT_s")
        kT_s = big.tile([P, BS], BF, tag="kT_s")
        # Per-head V, ones-augmented: 64 s-tiles, each [128, 65] with col 64 == 1.0
        v_s = [
            big.tile([P, NST * 65], BF, tag=f"v{h}_s", name=f"v{h}_s")
            for h in range(2)
        ]
        for h in range(2):
            nc.gpsimd.memset(v_s[h][:], 1.0)

        xT_t = xT.rearrange("(kt p) t -> p kt t", p=P)
        kvT_t = kvT.rearrange("(kt p) s -> p kt s", p=P)

        def load_x_batch(b, split=1):
            # split>1: finer-grained loads so b0's q-proj starts as data lands
            x_t = xin.tile([P, KT, T], BF, tag="x_t", name="x_t")
            csz = T // split
            for c in range(split):
                sl = slice(c * csz, (c + 1) * csz)
                for kt in range(KT):
                    nc.sync.dma_start(
                        x_t[:, kt, sl],
                        xT_t[:, kt, b * T + c * csz: b * T + (c + 1) * csz],
                    )
            return x_t

        def load_kv_batch(b, split=1):
            kv_t = xin.tile([P, KT, S], BF, tag="kv_t", name="kv_t")
            csz = S // split
            for c in range(split):
                sl = slice(c * csz, (c + 1) * csz)
                for kt in range(KT):
                    nc.sync.dma_start(
                        kv_t[:, kt, sl],
                        kvT_t[:, kt, b * S + c * csz: b * S + (c + 1) * csz],
                    )
            return kv_t

        def q_proj_frags(x_t, b, half):
            # q projection for one 512-wide chunk, split into 2 fragments
            # (4 k-tiles each) so it can be spread across attention j-iters
            ch = 2 * b + half
            state = {}

            def frag(kts):
                def run():
                    if "ps" not in state:
                        state["ps"] = op_pool.tile([P, TCH], F32, tag="op", name="qps")
                    ps = state["ps"]
                    for kt in kts:
                        nc.tensor.matmul(
                            ps[:], wq_s[:, kt, :],
                            x_t[:, kt, half * TCH:(half + 1) * TCH],
                            start=(kt == 0), stop=(kt == KT - 1),
                        )
                    if KT - 1 in kts:
                        nc.vector.tensor_copy(
                            qT_s[:, ch * TCH:(ch + 1) * TCH], ps[:]
                        )
                return run

            return [frag(range(0, 4)), frag(range(4, KT))]

        def kv_proj_frags(kv_t, b, quarter):
            # kT projection (1 fragment) + V projection (2 fragments) for one
            # 512-wide kv chunk
            ch = 4 * b + quarter
            q0 = quarter * TCH
            state = {}

            def k_frag():
                ps = op_pool.tile([P, TCH], F32, tag="op", name="kps")
                for kt in range(KT):
                    nc.tensor.matmul(
                        ps[:], wk_s[:, kt, :], kv_t[:, kt, q0:q0 + TCH],
                        start=(kt == 0), stop=(kt == KT - 1),
                    )
                nc.vector.tensor_copy(kT_s[:, ch * TCH:(ch + 1) * TCH], ps[:])

            # V projection: [s, c] orientation, 4 s-subtiles share one bank.
            # start only on the bank's first matmul: start=True marks the
            # whole 2KB zero-region pending-zero, so later subtiles' first
            # writes overwrite (not accumulate) stale data automatically.
            def v_frag(kts):
                def run():
                    if "vps" not in state:
                        state["vps"] = op_pool.tile(
                            [P, 4, P], F32, tag="op", name="vps"
                        )
                    vps = state["vps"]
                    for kt in kts:
                        for sub in range(4):
                            nc.tensor.matmul(
                                vps[:, sub, :],
                                kv_t[:, kt, q0 + sub * P:q0 + (sub + 1) * P],
                                wv_s[:, kt, :],
                                start=(kt == 0 and sub == 0),
                                stop=(kt == KT - 1 and sub == 3),
                            )
                    if KT - 1 in kts:
                        for sub in range(4):
                            jg = ch * 4 + sub
                            nc.vector.tensor_copy(
                                v_s[0][:, jg * 65:jg * 65 + 64], vps[:, sub, 0:64]
                            )
                            nc.vector.tensor_copy(
                                v_s[1][:, jg * 65:jg * 65 + 64], vps[:, sub, 64:128]
                            )
                return run

            return [k_frag, v_frag(range(0, 4)), v_frag(range(4, KT))]

        def attention_block(b, t2, fillers=()):
            # fillers: [(j, fn)] — PE filler work (next batch's projections,
            # previous block's o-proj) issued after iteration j so the tensor
            # engine never idles long enough for the HAM clock gate to
            # re-throttle. Returns o-proj closures to interleave into the
            # NEXT block.
            fmap = {}
            for j, fn in fillers:
                fmap.setdefault(j, []).append(fn)
            t0 = b * T + t2 * TCH
            ats = [
                at_pool.tile([65, TCH], F32, tag="at", name=f"at{h}")
                for h in range(2)
            ]
            for j in range(NJ):
                jg = b * NJ + j
                for fn in fmap.get(j, ()):
                    fn()
                sc = mm_ps.tile([P, 1024], F32, tag="mm", name="sc")
                for h in range(2):
                    hp = h * 64
                    nc.tensor.matmul(
                        sc[:, h * TCH:(h + 1) * TCH],
                        kT_s[hp:hp + 64, b * S + j * P: b * S + (j + 1) * P],
                        qT_s[hp:hp + 64, t0:t0 + TCH],
                        start=True, stop=True,
                    )
                pt = ptp.tile([P, 1024], BF, tag="pt", name="pt")
                nc.scalar.activation(
                    pt[:], sc[:],
                    mybir.ActivationFunctionType.Exp,
                    scale=EXP_SCALE,
                )
                for h in range(2):
                    nc.tensor.matmul(
                        ats[h][:],
                        v_s[h][:, jg * 65:(jg + 1) * 65],
                        pt[:, h * TCH:(h + 1) * TCH],
                        start=(j == 0), stop=(j == NJ - 1),
                    )

            # --- normalization (inline: frees the at PSUM banks fast) ---
            # rowsums -> SBUF partition 0 first (custom-DVE ops misread
            # PSUM/base-64 sources on HW; plain casts handle them fine).
            # Chain order matters: recip/broadcast first so the gpsimd
            # broadcast (~3.4us incl. queue drain) overlaps the a_bf casts.
            rsum = smalls.tile([1, 2, TCH], F32, tag="rsum", name="rsum")
            for h in range(2):
                nc.vector.tensor_copy(rsum[0:1, h, :], ats[h][64:65, :])
            # approx-reciprocal (51 ULP, plenty for softmax; rowsums are in
            # [~2, ~2e6] so no edge cases)
            rinv = smalls.tile([1, 2, TCH], F32, tag="rinv", name="rinv")
            nc.vector.reciprocal_approx_fast(rinv[0:1, :, :], rsum[0:1, :, :])
            rinv_bf = smalls.tile([1, 2, TCH], BF, tag="rinvbf", name="rinv_bf")
            nc.vector.tensor_copy(rinv_bf[:], rinv[:])
            # unnormalized attention out -> SBUF bf16 (both heads packed)
            a_bf = atsb.tile([P, TCH], BF, tag="abf", name="a_bf")
            for h in range(2):
                nc.vector.tensor_copy(a_bf[h * 64:(h + 1) * 64, :], ats[h][0:64, :])
            # broadcast rinv across all partitions on the (idle) gpsimd
            # engine; row p of rbs = [rinv_h0 | rinv_h1] (1KB bf16/partition)
            rbs = atsb.tile([P, 2, TCH], BF, tag="rbs", name="rbs")
            nc.gpsimd.partition_broadcast(rbs[:], rinv_bf[0:1, :, :])
            # aT = a_bf * rinv  (normalized, bf16, o-proj stationary layout)
            aT = atsb.tile([P, TCH], BF, tag="aT", name="aT")
            for h in range(2):
                nc.vector.scalar_tensor_tensor(
                    aT[h * 64:(h + 1) * 64, :],
                    a_bf[h * 64:(h + 1) * 64, :], 1.0,
                    rbs[h * 64:(h + 1) * 64, h, :],
                    mybir.AluOpType.bypass, mybir.AluOpType.mult,
                )

            # --- o-proj closures (deferred into the next block as filler) ---
            def oproj_sub(sub):
                def run():
                    ot = outp.tile([P, D], BF, tag="ot", name="ot")
                    for n in range(D // TCH):  # 2
                        ops = op_pool.tile([P, TCH], F32, tag="op", name="ops")
                        nc.tensor.matmul(
                            ops[:],
                            aT[:, sub * P:(sub + 1) * P],
                            wo_s[:, n * TCH:(n + 1) * TCH],
                            start=True, stop=True,
                        )
                        nc.vector.tensor_copy(ot[:, n * TCH:(n + 1) * TCH], ops[:])
                    nc.sync.dma_start(out[t0 + sub * P:t0 + (sub + 1) * P, :], ot[:])
                return run

            return [oproj_sub(sub) for sub in range(4)]

        # PE warmup: throwaway matmuls on the (tiny, fast-loading) weight
        # tiles engage the HAM clock gate to 8/8 during the DMA lead-in,
        # so the first real matmuls run at 2.4 GHz. Also pre-load the ACT
        # exp table so the first real exp doesn't pay the ~2.7us load.
        warm_sb = smalls.tile([1, 1], F32, tag="rinv", name="warm_sb")
        nc.scalar.activation(
            warm_sb[:], ones1[:], mybir.ActivationFunctionType.Exp
        )
        warm_ps = op_pool.tile([P, TCH], F32, tag="op", name="warm_ps")
        wq_flat = wq_s.rearrange("p k c -> p (k c)")
        for i in range(24):
            nc.tensor.matmul(
                warm_ps[:], wq_s[:, i % KT, :], wq_flat[:, :TCH],
                start=True, stop=True,
            )

        # b-major pipeline: batch 0 projects upfront; batch b+1's
        # projections are interleaved into batch b's attention as PE filler,
        # fragmented so no single j-iter gets a large PE burst (keeps the
        # ACT pipeline fed and the HAM clock gate warm). Each block also
        # carries the previous block's o-proj as filler.
        x_t = load_x_batch(0, split=2)
        kv_t = load_kv_batch(0, split=4)
        for fn in q_proj_frags(x_t, 0, 0) + q_proj_frags(x_t, 0, 1):
            fn()
        for quarter in range(4):
            for fn in kv_proj_frags(kv_t, 0, quarter):
                fn()
        otail = []
        for b in range(B):
            if b + 1 < B:
                nx_t = load_x_batch(b + 1)
                nkv_t = load_kv_batch(b + 1)
                frags0 = (
                    q_proj_frags(nx_t, b + 1, 0)
                    + q_proj_frags(nx_t, b + 1, 1)
                    + kv_proj_frags(nkv_t, b + 1, 0)
                )
                frags1 = (
                    kv_proj_frags(nkv_t, b + 1, 1)
                    + kv_proj_frags(nkv_t, b + 1, 2)
                    + kv_proj_frags(nkv_t, b + 1, 3)
                )
            else:
                frags0, frags1 = [], []
            # o-proj of the previous block goes late (j=5,8,11,14): its aT
            # depends on the boundary normalization chain (~4.5us of DVE +
            # gpsimd latency); placing it early stalls the in-order PE queue.
            f0 = list(zip((5, 8, 11, 14), otail)) + list(enumerate(frags0, start=1))
            otail = attention_block(b, 0, f0)
            f1 = list(zip((5, 8, 11, 14), otail)) + list(enumerate(frags1, start=1))
            otail = attention_block(b, 1, f1)
        for fn in otail:
            fn()

    nc.compile()
    return nc


_NC_CACHE = None


def _get_nc():
    global _NC_CACHE
    if _NC_CACHE is None:
        _NC_CACHE = build_nc()
    return _NC_CACHE


def make_in_maps(query, key_value, wq, wk, wv, wo):
    q2 = np.ascontiguousarray(np.asarray(query, np.float32).reshape(BT, D))
    kv2 = np.ascontiguousarray(np.asarray(key_value, np.float32).reshape(BS, D))
    xT = np.ascontiguousarray(q2.astype(NPBF).T)
    kvT = np.ascontiguousarray(kv2.astype(NPBF).T)
    wq = np.asarray(wq, np.float32)
    wk = np.asarray(wk, np.float32)
    wv = np.asarray(wv, np.float32)
    wo = np.asarray(wo, np.float32)
    in_maps = []
    for c in range(NCORES):
        cs = slice(c * P, (c + 1) * P)
        in_maps.append({
            "xT": xT,
            "kvT": kvT,
            "wqT": np.ascontiguousarray(wq[cs, :].astype(NPBF).T),
            "wkT": np.ascontiguousarray(wk[cs, :].astype(NPBF).T),
            "wvT": np.ascontiguousarray(wv[cs, :].astype(NPBF).T),
            "woT": np.ascontiguousarray(wo[:, cs].astype(NPBF).T),
            "eye2": np.eye(2, dtype=np.float32),
        })
    return in_maps


def run(inputs, trace=False, **kwargs):
    from concourse.bass_utils import run_bass_kernel_spmd

    nc = _get_nc()
    in_maps = make_in_maps(**inputs)
    res = run_bass_kernel_spmd(
        nc, in_maps, core_ids=list(range(NCORES)), trace=trace, **kwargs
    )
    acc = np.zeros((BT, D), np.float64)
    for r in res.results:
        acc += r["out"].astype(np.float64)
    return acc.astype(np.float32).reshape(B, T, D), res


def kernel(**inputs):
    return run(inputs, trace=False)[0]


# revision 19
# speedup vs baseline: 1.4102x; 1.0525x over previous
"""Trainium2 Bass kernel for nn_CrossAttention (B=4, T=1024, S=2048, D=1024, H=16).

Sharding: tensor-parallel over heads. Each of the 8 cores owns 2 heads
(a 128-column slice of the q/k/v projections and the matching 128-row
slice of the o-projection input). Every core computes a full-shape
partial o-proj output (bf16); the host sums the 8 partials (the
all-reduce is done on the host during the gather/unshard step).

Layout strategy: all device matmuls contract along the SBUF partition
axis, so the host pre-transposes the activations and weights (free on
host, removes every on-chip transpose):
  xT  [D, B*T]  = query^T          (bf16)
  kvT [D, B*S]  = key_value^T      (bf16)
  wqT/wkT/wvT [D, 128] per core    (bf16)
  woT [128, D] per core            (bf16)

Pipeline per core (all matmul accumulation in fp32 PSUM):
  qT = WqT.T @ xT            -> [128c, B*T]   (c on partitions)
  kT = WkT.T @ kvT           -> [128c, B*S]
  V  = kvT.T @ WvT           -> [s, c] tiles, stored ones-augmented [128, 65]
  scoresT = kT.T @ qT per head (K=64)         -> [128s, 512t] PSUM
  PT = exp(0.125 * scoresT)  (ACT, no max-subtraction: |scores| < ~7)
  attnT[h] += V_aug.T @ PT   -> [65, 512t]; row 64 = softmax rowsum (free)
  rinv = 1/rowsum (DVE) -> bf16 [1, 512]; rb[h] = ones64.T @ rinv
    (K=1 PE outer product broadcasts rinv across 64 partitions in PSUM)
  aT[h] = attnT[h] * rb[h]   (DVE scalar_tensor_tensor, bf16 out)
  o-proj: out[128t, d] = aT[:, tsub].T @ woT  (K=128, fp32 PSUM)
  out partial stored bf16; o-proj deferred into the next attention
  block's j-loop as tensor-engine filler (keeps the HAM clock gate at
  full speed).
"""

import os
import sys
from contextlib import ExitStack

import numpy as np

for _p in (
    "/root/.axon_site",
    "/root/.axon_site/_ro/trn_rl_repo",
    "/root/.axon_site/_ro/pypackages",
    "/opt/trn_rl_repo",
):
    if os.path.isdir(_p) and _p not in sys.path:
        sys.path.append(_p)

import ml_dtypes  # noqa: E402

import concourse.bass as bass  # noqa: E402
import concourse.mybir as mybir  # noqa: E402
import concourse.tile as tile  # noqa: E402
from concourse import bacc  # noqa: E402

BF = mybir.dt.bfloat16
F32 = mybir.dt.float32
NPBF = ml_dtypes.bfloat16

B, T, S, D = 4, 1024, 2048, 1024
BT, BS = B * T, B * S
P = 128
NCORES = 8
KT = D // P          # 8 contraction tiles of 128
TCH = 512            # free-dim chunk for projections / attention t-chunks
NJ = S // P          # 16 s-tiles of 128 per batch
NST = BS // P        # 64 s-tiles total
EXP_SCALE = float(64 ** -0.5)  # folded into the ACT exp


def build_nc():
    nc = bacc.Bacc("TRN2", target_bir_lowering=False)

    xT = nc.dram_tensor("xT", [D, BT], BF, kind="ExternalInput").ap()
    kvT = nc.dram_tensor("kvT", [D, BS], BF, kind="ExternalInput").ap()
    wqT = nc.dram_tensor("wqT", [D, P], BF, kind="ExternalInput").ap()
    wkT = nc.dram_tensor("wkT", [D, P], BF, kind="ExternalInput").ap()
    wvT = nc.dram_tensor("wvT", [D, P], BF, kind="ExternalInput").ap()
    woT = nc.dram_tensor("woT", [P, D], BF, kind="ExternalInput").ap()
    eye2_d = nc.dram_tensor("eye2", [2, 2], F32, kind="ExternalInput").ap()
    out = nc.dram_tensor("out", [BT, D], BF, kind="ExternalOutput").ap()

    with tile.TileContext(nc) as tc, ExitStack() as ctx:
        consts = ctx.enter_context(tc.tile_pool(name="consts", bufs=1))
        big = ctx.enter_context(tc.tile_pool(name="big", bufs=1))
        xin = ctx.enter_context(tc.tile_pool(name="xin", bufs=2))
        ptp = ctx.enter_context(tc.tile_pool(name="ptp", bufs=4))
        atsb = ctx.enter_context(tc.tile_pool(name="atsb", bufs=6))
        smalls = ctx.enter_context(tc.tile_pool(name="smalls", bufs=2))
        outp = ctx.enter_context(tc.tile_pool(name="outp", bufs=3))
        # PSUM budget (8 banks): mm [128,1024]x2 = 4 + at 2 + op 2
        mm_ps = ctx.enter_context(tc.tile_pool(name="mm_ps", bufs=2, space="PSUM"))
        at_pool = ctx.enter_context(tc.tile_pool(name="at_ps", bufs=2, space="PSUM"))
        op_pool = ctx.enter_context(tc.tile_pool(name="op_ps", bufs=2, space="PSUM"))

        # ---- resident weights ----
        wq_s = consts.tile([P, KT, P], BF, tag="wq_s")
        wk_s = consts.tile([P, KT, P], BF, tag="wk_s")
        wv_s = consts.tile([P, KT, P], BF, tag="wv_s")
        wqT_t = wqT.rearrange("(kt p) c -> p kt c", p=P)
        wkT_t = wkT.rearrange("(kt p) c -> p kt c", p=P)
        wvT_t = wvT.rearrange("(kt p) c -> p kt c", p=P)
        # one strided DMA per weight (sync-queue issue slots are ~0.6us each)
        nc.sync.dma_start(wq_s[:], wqT_t[:])
        nc.sync.dma_start(wk_s[:], wkT_t[:])
        nc.sync.dma_start(wv_s[:], wvT_t[:])
        wo_s = consts.tile([P, D], BF, tag="wo_s")
        nc.sync.dma_start(wo_s[:], woT)
        # [1,1] ones (fp32, ACT warmup)
        ones1 = consts.tile([1, 1], F32, tag="ones1")
        nc.sync.dma_start(ones1[:], eye2_d[0:1, 0:1])

        # ---- resident intermediates ----
        qT_s = big.tile([P, BT], BF, tag="qT_s")
        kT_s = big.tile([P, BS], BF, tag="kT_s")
        # Per-head V, ones-augmented: 64 s-tiles, each [128, 65] with col 64 == 1.0
        v_s = [
            big.tile([P, NST * 65], BF, tag=f"v{h}_s", name=f"v{h}_s")
            for h in range(2)
        ]
        for h in range(2):
            nc.gpsimd.memset(v_s[h][:], 1.0)

        xT_t = xT.rearrange("(kt p) t -> p kt t", p=P)
        kvT_t = kvT.rearrange("(kt p) s -> p kt s", p=P)

        def load_x_batch(b, split=1):
            # one strided DMA per split-chunk (all kt at once); split>1 only
            # for b0 so the lead-in projections start as data lands
            x_t = xin.tile([P, KT, T], BF, tag="x_t", name="x_t")
            csz = T // split
            for c in range(split):
                nc.sync.dma_start(
                    x_t[:, :, c * csz:(c + 1) * csz],
                    xT_t[:, :, b * T + c * csz: b * T + (c + 1) * csz],
                )
            return x_t

        def load_kv_batch(b, split=1):
            kv_t = xin.tile([P, KT, S], BF, tag="kv_t", name="kv_t")
            csz = S // split
            for c in range(split):
                nc.sync.dma_start(
                    kv_t[:, :, c * csz:(c + 1) * csz],
                    kvT_t[:, :, b * S + c * csz: b * S + (c + 1) * csz],
                )
            return kv_t

        def q_proj_frags(x_t, b, half):
            # q projection for one 512-wide chunk, split into 2 fragments
            # (4 k-tiles each) so it can be spread across attention j-iters
            ch = 2 * b + half
            state = {}

            def frag(kts):
                def run():
                    if "ps" not in state:
                        state["ps"] = op_pool.tile([P, TCH], F32, tag="op", name="qps")
                    ps = state["ps"]
                    for kt in kts:
                        nc.tensor.matmul(
                            ps[:], wq_s[:, kt, :],
                            x_t[:, kt, half * TCH:(half + 1) * TCH],
                            start=(kt == 0), stop=(kt == KT - 1),
                        )
                    if KT - 1 in kts:
                        nc.vector.tensor_copy(
                            qT_s[:, ch * TCH:(ch + 1) * TCH], ps[:]
                        )
                return run

            return [frag(range(0, 4)), frag(range(4, KT))]

        def kv_proj_frags(kv_t, b, quarter):
            # kT projection (1 fragment) + V projection (2 fragments) for one
            # 512-wide kv chunk
            ch = 4 * b + quarter
            q0 = quarter * TCH
            state = {}

            def k_frag():
                ps = op_pool.tile([P, TCH], F32, tag="op", name="kps")
                for kt in range(KT):
                    nc.tensor.matmul(
                        ps[:], wk_s[:, kt, :], kv_t[:, kt, q0:q0 + TCH],
                        start=(kt == 0), stop=(kt == KT - 1),
                    )
                nc.vector.tensor_copy(kT_s[:, ch * TCH:(ch + 1) * TCH], ps[:])

            # V projection: [s, c] orientation, 4 s-subtiles share one bank.
            # start only on the bank's first matmul: start=True marks the
            # whole 2KB zero-region pending-zero, so later subtiles' first
            # writes overwrite (not accumulate) stale data automatically.
            def v_frag(kts):
                def run():
                    if "vps" not in state:
                        state["vps"] = op_pool.tile(
                            [P, 4, P], F32, tag="op", name="vps"
                        )
                    vps = state["vps"]
                    for kt in kts:
                        for sub in range(4):
                            nc.tensor.matmul(
                                vps[:, sub, :],
                                kv_t[:, kt, q0 + sub * P:q0 + (sub + 1) * P],
                                wv_s[:, kt, :],
                                start=(kt == 0 and sub == 0),
                                stop=(kt == KT - 1 and sub == 3),
                            )
                    if KT - 1 in kts:
                        for sub in range(4):
                            jg = ch * 4 + sub
                            nc.vector.tensor_copy(
                                v_s[0][:, jg * 65:jg * 65 + 64], vps[:, sub, 0:64]
                            )
                            nc.vector.tensor_copy(
                                v_s[1][:, jg * 65:jg * 65 + 64], vps[:, sub, 64:128]
                            )
                return run

            return [k_frag, v_frag(range(0, 4)), v_frag(range(4, KT))]

        def attention_block(b, t2, fillers=()):
            # fillers: [(j, fn)] — PE filler work (next batch's projections,
            # previous block's o-proj) issued after iteration j so the tensor
            # engine never idles long enough for the HAM clock gate to
            # re-throttle. Returns o-proj closures to interleave into the
            # NEXT block.
            fmap = {}
            for j, fn in fillers:
                fmap.setdefault(j, []).append(fn)
            t0 = b * T + t2 * TCH
            ats = [
                at_pool.tile([65, TCH], F32, tag="at", name=f"at{h}")
                for h in range(2)
            ]
            for j in range(NJ):
                jg = b * NJ + j
                for fn in fmap.get(j, ()):
                    fn()
                sc = mm_ps.tile([P, 1024], F32, tag="mm", name="sc")
                for h in range(2):
                    hp = h * 64
                    nc.tensor.matmul(
                        sc[:, h * TCH:(h + 1) * TCH],
                        kT_s[hp:hp + 64, b * S + j * P: b * S + (j + 1) * P],
                        qT_s[hp:hp + 64, t0:t0 + TCH],
                        start=True, stop=True,
                    )
                pt = ptp.tile([P, 1024], BF, tag="pt", name="pt")
                nc.scalar.activation(
                    pt[:], sc[:],
                    mybir.ActivationFunctionType.Exp,
                    scale=EXP_SCALE,
                )
                for h in range(2):
                    nc.tensor.matmul(
                        ats[h][:],
                        v_s[h][:, jg * 65:(jg + 1) * 65],
                        pt[:, h * TCH:(h + 1) * TCH],
                        start=(j == 0), stop=(j == NJ - 1),
                    )

            # --- normalization (inline: frees the at PSUM banks fast) ---
            # rowsums -> SBUF partition 0 first (custom-DVE ops misread
            # PSUM/base-64 sources on HW; plain casts handle them fine).
            # Chain order matters: recip/broadcast first so the gpsimd
            # broadcast (~3.4us incl. queue drain) overlaps the a_bf casts.
            rsum = smalls.tile([1, 2, TCH], F32, tag="rsum", name="rsum")
            for h in range(2):
                nc.vector.tensor_copy(rsum[0:1, h, :], ats[h][64:65, :])
            # approx-reciprocal (51 ULP, plenty for softmax; rowsums are in
            # [~2, ~2e6] so no edge cases)
            rinv = smalls.tile([1, 2, TCH], F32, tag="rinv", name="rinv")
            nc.vector.reciprocal_approx_fast(rinv[0:1, :, :], rsum[0:1, :, :])
            rinv_bf = smalls.tile([1, 2, TCH], BF, tag="rinvbf", name="rinv_bf")
            nc.vector.tensor_copy(rinv_bf[:], rinv[:])
            # unnormalized attention out -> SBUF bf16 (both heads packed)
            a_bf = atsb.tile([P, TCH], BF, tag="abf", name="a_bf")
            for h in range(2):
                nc.vector.tensor_copy(a_bf[h * 64:(h + 1) * 64, :], ats[h][0:64, :])
            # broadcast rinv across all partitions on the (idle) gpsimd
            # engine; row p of rbs = [rinv_h0 | rinv_h1] (1KB bf16/partition)
            rbs = atsb.tile([P, 2, TCH], BF, tag="rbs", name="rbs")
            nc.gpsimd.partition_broadcast(rbs[:], rinv_bf[0:1, :, :])
            # aT = a_bf * rinv  (normalized, bf16, o-proj stationary layout)
            aT = atsb.tile([P, TCH], BF, tag="aT", name="aT")
            for h in range(2):
                nc.vector.scalar_tensor_tensor(
                    aT[h * 64:(h + 1) * 64, :],
                    a_bf[h * 64:(h + 1) * 64, :], 1.0,
                    rbs[h * 64:(h + 1) * 64, h, :],
                    mybir.AluOpType.bypass, mybir.AluOpType.mult,
                )

            # --- o-proj closures (deferred into the next block as filler) ---
            def oproj_sub(sub):
                def run():
                    ot = outp.tile([P, D], BF, tag="ot", name="ot")
                    for n in range(D // TCH):  # 2
                        ops = op_pool.tile([P, TCH], F32, tag="op", name="ops")
                        nc.tensor.matmul(
                            ops[:],
                            aT[:, sub * P:(sub + 1) * P],
                            wo_s[:, n * TCH:(n + 1) * TCH],
                            start=True, stop=True,
                        )
                        nc.vector.tensor_copy(ot[:, n * TCH:(n + 1) * TCH], ops[:])
                    nc.sync.dma_start(out[t0 + sub * P:t0 + (sub + 1) * P, :], ot[:])
                return run

            return [oproj_sub(sub) for sub in range(4)]

        # PE warmup: throwaway matmuls on the (tiny, fast-loading) weight
        # tiles engage the HAM clock gate to 8/8 during the DMA lead-in,
        # so the first real matmuls run at 2.4 GHz. Also pre-load the ACT
        # exp table so the first real exp doesn't pay the ~2.7us load.
        warm_sb = smalls.tile([1, 1], F32, tag="rinv", name="warm_sb")
        nc.scalar.activation(
            warm_sb[:], ones1[:], mybir.ActivationFunctionType.Exp
        )
        warm_ps = op_pool.tile([P, TCH], F32, tag="op", name="warm_ps")
        wq_flat = wq_s.rearrange("p k c -> p (k c)")
        for i in range(24):
            nc.tensor.matmul(
                warm_ps[:], wq_s[:, i % KT, :], wq_flat[:, :TCH],
                start=True, stop=True,
            )

        # b-major pipeline: batch 0 projects upfront; batch b+1's
        # projections are interleaved into batch b's attention as PE filler,
        # fragmented so no single j-iter gets a large PE burst (keeps the
        # ACT pipeline fed and the HAM clock gate warm). Each block also
        # carries the previous block's o-proj as filler.
        x_t = load_x_batch(0, split=2)
        kv_t = load_kv_batch(0, split=4)
        for fn in q_proj_frags(x_t, 0, 0) + q_proj_frags(x_t, 0, 1):
            fn()
        for quarter in range(4):
            for fn in kv_proj_frags(kv_t, 0, quarter):
                fn()
        otail = []
        for b in range(B):
            if b + 1 < B:
                nx_t = load_x_batch(b + 1)
                nkv_t = load_kv_batch(b + 1)
                frags0 = (
                    q_proj_frags(nx_t, b + 1, 0)
                    + q_proj_frags(nx_t, b + 1, 1)
                    + kv_proj_frags(nkv_t, b + 1, 0)
                )
                frags1 = (
                    kv_proj_frags(nkv_t, b + 1, 1)
                    + kv_proj_frags(nkv_t, b + 1, 2)
                    + kv_proj_frags(nkv_t, b + 1, 3)
                )
            else:
                frags0, frags1 = [], []
            # o-proj of the previous block goes late (j=5,8,11,14): its aT
            # depends on the boundary normalization chain (~4.5us of DVE +
            # gpsimd latency); placing it early stalls the in-order PE queue.
            f0 = list(zip((5, 8, 11, 14), otail)) + list(enumerate(frags0, start=1))
            otail = attention_block(b, 0, f0)
            f1 = list(zip((5, 8, 11, 14), otail)) + list(enumerate(frags1, start=1))
            otail = attention_block(b, 1, f1)
        for fn in otail:
            fn()

    nc.compile()
    return nc


_NC_CACHE = None


def _get_nc():
    global _NC_CACHE
    if _NC_CACHE is None:
        _NC_CACHE = build_nc()
    return _NC_CACHE


def make_in_maps(query, key_value, wq, wk, wv, wo):
    q2 = np.ascontiguousarray(np.asarray(query, np.float32).reshape(BT, D))
    kv2 = np.ascontiguousarray(np.asarray(key_value, np.float32).reshape(BS, D))
    xT = np.ascontiguousarray(q2.astype(NPBF).T)
    kvT = np.ascontiguousarray(kv2.astype(NPBF).T)
    wq = np.asarray(wq, np.float32)
    wk = np.asarray(wk, np.float32)
    wv = np.asarray(wv, np.float32)
    wo = np.asarray(wo, np.float32)
    in_maps = []
    for c in range(NCORES):
        cs = slice(c * P, (c + 1) * P)
        in_maps.append({
            "xT": xT,
            "kvT": kvT,
            "wqT": np.ascontiguousarray(wq[cs, :].astype(NPBF).T),
            "wkT": np.ascontiguousarray(wk[cs, :].astype(NPBF).T),
            "wvT": np.ascontiguousarray(wv[cs, :].astype(NPBF).T),
            "woT": np.ascontiguousarray(wo[:, cs].astype(NPBF).T),
            "eye2": np.eye(2, dtype=np.float32),
        })
    return in_maps


def run(inputs, trace=False, **kwargs):
    from concourse.bass_utils import run_bass_kernel_spmd

    nc = _get_nc()
    in_maps = make_in_maps(**inputs)
    res = run_bass_kernel_spmd(
        nc, in_maps, core_ids=list(range(NCORES)), trace=trace, **kwargs
    )
    acc = np.zeros((BT, D), np.float64)
    for r in res.results:
        acc += r["out"].astype(np.float64)
    return acc.astype(np.float32).reshape(B, T, D), res


def kernel(**inputs):
    return run(inputs, trace=False)[0]


# revision 20
# speedup vs baseline: 1.4733x; 1.0447x over previous
"""Trainium2 Bass kernel for nn_CrossAttention (B=4, T=1024, S=2048, D=1024, H=16).

Sharding: tensor-parallel over heads. Each of the 8 cores owns 2 heads
(a 128-column slice of the q/k/v projections and the matching 128-row
slice of the o-projection input). Every core computes a full-shape
partial o-proj output (bf16); the host sums the 8 partials (the
all-reduce is done on the host during the gather/unshard step).

Layout strategy: all device matmuls contract along the SBUF partition
axis, so the host pre-transposes the activations and weights (free on
host, removes every on-chip transpose):
  xT  [D, B*T]  = query^T          (bf16)
  kvT [D, B*S]  = key_value^T      (bf16)
  wqT/wkT/wvT [D, 128] per core    (bf16)
  woT [128, D] per core            (bf16)

Pipeline per core (all matmul accumulation in fp32 PSUM):
  qT = WqT.T @ xT            -> [128c, B*T]   (c on partitions)
  kT = WkT.T @ kvT           -> [128c, B*S]
  V  = kvT.T @ WvT           -> [s, c] tiles, stored ones-augmented [128, 65]
  scoresT = kT.T @ qT per head (K=64)         -> [128s, 512t] PSUM
  PT = exp(0.125 * scoresT)  (ACT, no max-subtraction: |scores| < ~7)
  attnT[h] += V_aug.T @ PT   -> [65, 512t]; row 64 = softmax rowsum (free)
  rinv = 1/rowsum (DVE) -> bf16 [1, 512]; rb[h] = ones64.T @ rinv
    (K=1 PE outer product broadcasts rinv across 64 partitions in PSUM)
  aT[h] = attnT[h] * rb[h]   (DVE scalar_tensor_tensor, bf16 out)
  o-proj: out[128t, d] = aT[:, tsub].T @ woT  (K=128, fp32 PSUM)
  out partial stored bf16; o-proj deferred into the next attention
  block's j-loop as tensor-engine filler (keeps the HAM clock gate at
  full speed).
"""

import os
import sys
from contextlib import ExitStack

import numpy as np

for _p in (
    "/root/.axon_site",
    "/root/.axon_site/_ro/trn_rl_repo",
    "/root/.axon_site/_ro/pypackages",
    "/opt/trn_rl_repo",
):
    if os.path.isdir(_p) and _p not in sys.path:
        sys.path.append(_p)

import ml_dtypes  # noqa: E402

import concourse.bass as bass  # noqa: E402
import concourse.mybir as mybir  # noqa: E402
import concourse.tile as tile  # noqa: E402
from concourse import bacc  # noqa: E402

BF = mybir.dt.bfloat16
F32 = mybir.dt.float32
NPBF = ml_dtypes.bfloat16

B, T, S, D = 4, 1024, 2048, 1024
BT, BS = B * T, B * S
P = 128
NCORES = 8
KT = D // P          # 8 contraction tiles of 128
TCH = 512            # free-dim chunk for projections / attention t-chunks
NJ = S // P          # 16 s-tiles of 128 per batch
NST = BS // P        # 64 s-tiles total
EXP_SCALE = float(64 ** -0.5)  # folded into the ACT exp


def build_nc():
    nc = bacc.Bacc("TRN2", target_bir_lowering=False)

    xT = nc.dram_tensor("xT", [D, BT], BF, kind="ExternalInput").ap()
    kvT = nc.dram_tensor("kvT", [D, BS], BF, kind="ExternalInput").ap()
    wqT = nc.dram_tensor("wqT", [D, P], BF, kind="ExternalInput").ap()
    wkT = nc.dram_tensor("wkT", [D, P], BF, kind="ExternalInput").ap()
    wvT = nc.dram_tensor("wvT", [D, P], BF, kind="ExternalInput").ap()
    woT = nc.dram_tensor("woT", [P, D], BF, kind="ExternalInput").ap()
    eye2_d = nc.dram_tensor("eye2", [2, 2], F32, kind="ExternalInput").ap()
    out = nc.dram_tensor("out", [BT, D], BF, kind="ExternalOutput").ap()

    with tile.TileContext(nc) as tc, ExitStack() as ctx:
        consts = ctx.enter_context(tc.tile_pool(name="consts", bufs=1))
        big = ctx.enter_context(tc.tile_pool(name="big", bufs=1))
        xin = ctx.enter_context(tc.tile_pool(name="xin", bufs=2))
        ptp = ctx.enter_context(tc.tile_pool(name="ptp", bufs=4))
        atsb = ctx.enter_context(tc.tile_pool(name="atsb", bufs=6))
        smalls = ctx.enter_context(tc.tile_pool(name="smalls", bufs=2))
        outp = ctx.enter_context(tc.tile_pool(name="outp", bufs=3))
        # PSUM budget (8 banks): mm [128,1024]x2 = 4 + at 2 + op 2
        mm_ps = ctx.enter_context(tc.tile_pool(name="mm_ps", bufs=2, space="PSUM"))
        at_pool = ctx.enter_context(tc.tile_pool(name="at_ps", bufs=2, space="PSUM"))
        op_pool = ctx.enter_context(tc.tile_pool(name="op_ps", bufs=2, space="PSUM"))

        # ---- resident weights ----
        wq_s = consts.tile([P, KT, P], BF, tag="wq_s")
        wk_s = consts.tile([P, KT, P], BF, tag="wk_s")
        wv_s = consts.tile([P, KT, P], BF, tag="wv_s")
        wqT_t = wqT.rearrange("(kt p) c -> p kt c", p=P)
        wkT_t = wkT.rearrange("(kt p) c -> p kt c", p=P)
        wvT_t = wvT.rearrange("(kt p) c -> p kt c", p=P)
        # one strided DMA per weight (sync-queue issue slots are ~0.6us each)
        nc.sync.dma_start(wq_s[:], wqT_t[:])
        nc.sync.dma_start(wk_s[:], wkT_t[:])
        nc.sync.dma_start(wv_s[:], wvT_t[:])
        wo_s = consts.tile([P, D], BF, tag="wo_s")
        nc.sync.dma_start(wo_s[:], woT)
        # [1,1] ones (fp32, ACT warmup)
        ones1 = consts.tile([1, 1], F32, tag="ones1")
        nc.sync.dma_start(ones1[:], eye2_d[0:1, 0:1])

        # ---- resident intermediates ----
        qT_s = big.tile([P, BT], BF, tag="qT_s")
        kT_s = big.tile([P, BS], BF, tag="kT_s")
        # Per-head V, ones-augmented: 64 s-tiles, each [128, 65] with col 64 == 1.0
        v_s = [
            big.tile([P, NST * 65], BF, tag=f"v{h}_s", name=f"v{h}_s")
            for h in range(2)
        ]
        for h in range(2):
            nc.gpsimd.memset(v_s[h][:], 1.0)

        xT_t = xT.rearrange("(kt p) t -> p kt t", p=P)
        kvT_t = kvT.rearrange("(kt p) s -> p kt s", p=P)

        def load_x_batch(b, split=1):
            # one strided DMA per split-chunk (all kt at once); split>1 only
            # for b0 so the lead-in projections start as data lands
            x_t = xin.tile([P, KT, T], BF, tag="x_t", name="x_t")
            csz = T // split
            for c in range(split):
                nc.sync.dma_start(
                    x_t[:, :, c * csz:(c + 1) * csz],
                    xT_t[:, :, b * T + c * csz: b * T + (c + 1) * csz],
                )
            return x_t

        def load_kv_batch(b, split=1):
            kv_t = xin.tile([P, KT, S], BF, tag="kv_t", name="kv_t")
            csz = S // split
            for c in range(split):
                nc.sync.dma_start(
                    kv_t[:, :, c * csz:(c + 1) * csz],
                    kvT_t[:, :, b * S + c * csz: b * S + (c + 1) * csz],
                )
            return kv_t

        def q_proj_frags(x_t, b, half):
            # q projection for one 512-wide chunk, split into 2 fragments
            # (4 k-tiles each) so it can be spread across attention j-iters
            ch = 2 * b + half
            state = {}

            def frag(kts):
                def run():
                    if "ps" not in state:
                        state["ps"] = op_pool.tile([P, TCH], F32, tag="op", name="qps")
                    ps = state["ps"]
                    for kt in kts:
                        nc.tensor.matmul(
                            ps[:], wq_s[:, kt, :],
                            x_t[:, kt, half * TCH:(half + 1) * TCH],
                            start=(kt == 0), stop=(kt == KT - 1),
                        )
                    if KT - 1 in kts:
                        nc.vector.tensor_copy(
                            qT_s[:, ch * TCH:(ch + 1) * TCH], ps[:]
                        )
                return run

            return [frag(range(0, 4)), frag(range(4, KT))]

        def kv_proj_frags(kv_t, b, quarter):
            # kT projection (1 fragment) + V projection (2 fragments) for one
            # 512-wide kv chunk
            ch = 4 * b + quarter
            q0 = quarter * TCH
            state = {}

            def k_frag():
                ps = op_pool.tile([P, TCH], F32, tag="op", name="kps")
                for kt in range(KT):
                    nc.tensor.matmul(
                        ps[:], wk_s[:, kt, :], kv_t[:, kt, q0:q0 + TCH],
                        start=(kt == 0), stop=(kt == KT - 1),
                    )
                nc.vector.tensor_copy(kT_s[:, ch * TCH:(ch + 1) * TCH], ps[:])

            # V projection: [s, c] orientation, 4 s-subtiles share one bank.
            # start only on the bank's first matmul: start=True marks the
            # whole 2KB zero-region pending-zero, so later subtiles' first
            # writes overwrite (not accumulate) stale data automatically.
            def v_frag(kts):
                def run():
                    if "vps" not in state:
                        state["vps"] = op_pool.tile(
                            [P, 4, P], F32, tag="op", name="vps"
                        )
                    vps = state["vps"]
                    for kt in kts:
                        for sub in range(4):
                            nc.tensor.matmul(
                                vps[:, sub, :],
                                kv_t[:, kt, q0 + sub * P:q0 + (sub + 1) * P],
                                wv_s[:, kt, :],
                                start=(kt == 0 and sub == 0),
                                stop=(kt == KT - 1 and sub == 3),
                            )
                    if KT - 1 in kts:
                        for sub in range(4):
                            jg = ch * 4 + sub
                            nc.vector.tensor_copy(
                                v_s[0][:, jg * 65:jg * 65 + 64], vps[:, sub, 0:64]
                            )
                            nc.vector.tensor_copy(
                                v_s[1][:, jg * 65:jg * 65 + 64], vps[:, sub, 64:128]
                            )
                return run

            return [k_frag, v_frag(range(0, 4)), v_frag(range(4, KT))]

        def attention_block(b, t2, fillers=()):
            # fillers: [(j, fn)] — PE filler work (next batch's projections,
            # previous block's o-proj) issued after iteration j so the tensor
            # engine never idles long enough for the HAM clock gate to
            # re-throttle. Returns o-proj closures to interleave into the
            # NEXT block.
            fmap = {}
            for j, fn in fillers:
                fmap.setdefault(j, []).append(fn)
            t0 = b * T + t2 * TCH
            ats = [
                at_pool.tile([65, TCH], F32, tag="at", name=f"at{h}")
                for h in range(2)
            ]
            for j in range(NJ):
                jg = b * NJ + j
                for fn in fmap.get(j, ()):
                    fn()
                sc = mm_ps.tile([P, 1024], F32, tag="mm", name="sc")
                for h in range(2):
                    hp = h * 64
                    nc.tensor.matmul(
                        sc[:, h * TCH:(h + 1) * TCH],
                        kT_s[hp:hp + 64, b * S + j * P: b * S + (j + 1) * P],
                        qT_s[hp:hp + 64, t0:t0 + TCH],
                        start=True, stop=True,
                    )
                pt = ptp.tile([P, 1024], BF, tag="pt", name="pt")
                nc.scalar.activation(
                    pt[:], sc[:],
                    mybir.ActivationFunctionType.Exp,
                    scale=EXP_SCALE,
                )
                for h in range(2):
                    nc.tensor.matmul(
                        ats[h][:],
                        v_s[h][:, jg * 65:(jg + 1) * 65],
                        pt[:, h * TCH:(h + 1) * TCH],
                        start=(j == 0), stop=(j == NJ - 1),
                    )

            # --- normalization (inline: frees the at PSUM banks fast) ---
            # rowsums -> SBUF partition 0 first (custom-DVE ops misread
            # PSUM/base-64 sources on HW; plain casts handle them fine).
            # Chain order matters: recip/broadcast first so the gpsimd
            # broadcast (~3.4us incl. queue drain) overlaps the a_bf casts.
            rsum = smalls.tile([1, 2, TCH], F32, tag="rsum", name="rsum")
            for h in range(2):
                nc.vector.tensor_copy(rsum[0:1, h, :], ats[h][64:65, :])
            # approx-reciprocal (51 ULP, plenty for softmax; rowsums are in
            # [~2, ~2e6] so no edge cases)
            rinv = smalls.tile([1, 2, TCH], F32, tag="rinv", name="rinv")
            nc.vector.reciprocal_approx_fast(rinv[0:1, :, :], rsum[0:1, :, :])
            rinv_bf = smalls.tile([1, 2, TCH], BF, tag="rinvbf", name="rinv_bf")
            nc.vector.tensor_copy(rinv_bf[:], rinv[:])
            # unnormalized attention out -> SBUF bf16 (both heads packed)
            a_bf = atsb.tile([P, TCH], BF, tag="abf", name="a_bf")
            for h in range(2):
                nc.vector.tensor_copy(a_bf[h * 64:(h + 1) * 64, :], ats[h][0:64, :])
            # broadcast rinv across all partitions on the (idle) gpsimd
            # engine; row p of rbs = [rinv_h0 | rinv_h1] (1KB bf16/partition)
            rbs = atsb.tile([P, 2, TCH], BF, tag="rbs", name="rbs")
            nc.gpsimd.partition_broadcast(rbs[:], rinv_bf[0:1, :, :])
            # aT = a_bf * rinv  (normalized, bf16, o-proj stationary layout)
            aT = atsb.tile([P, TCH], BF, tag="aT", name="aT")
            for h in range(2):
                nc.vector.scalar_tensor_tensor(
                    aT[h * 64:(h + 1) * 64, :],
                    a_bf[h * 64:(h + 1) * 64, :], 1.0,
                    rbs[h * 64:(h + 1) * 64, h, :],
                    mybir.AluOpType.bypass, mybir.AluOpType.mult,
                )

            # --- o-proj closures (deferred into the next block as filler) ---
            def oproj_sub(sub):
                def run():
                    ot = outp.tile([P, D], BF, tag="ot", name="ot")
                    for n in range(D // TCH):  # 2
                        ops = op_pool.tile([P, TCH], F32, tag="op", name="ops")
                        nc.tensor.matmul(
                            ops[:],
                            aT[:, sub * P:(sub + 1) * P],
                            wo_s[:, n * TCH:(n + 1) * TCH],
                            start=True, stop=True,
                        )
                        nc.vector.tensor_copy(ot[:, n * TCH:(n + 1) * TCH], ops[:])
                    nc.sync.dma_start(out[t0 + sub * P:t0 + (sub + 1) * P, :], ot[:])
                return run

            return [oproj_sub(sub) for sub in range(4)]

        # PE warmup: throwaway matmuls on the (tiny, fast-loading) weight
        # tiles engage the HAM clock gate to 8/8 during the DMA lead-in,
        # so the first real matmuls run at 2.4 GHz. Also pre-load the ACT
        # exp table so the first real exp doesn't pay the ~2.7us load.
        warm_sb = smalls.tile([1, 1], F32, tag="rinv", name="warm_sb")
        nc.scalar.activation(
            warm_sb[:], ones1[:], mybir.ActivationFunctionType.Exp
        )
        warm_ps = op_pool.tile([P, TCH], F32, tag="op", name="warm_ps")
        wq_flat = wq_s.rearrange("p k c -> p (k c)")
        for i in range(24):
            nc.tensor.matmul(
                warm_ps[:], wq_s[:, i % KT, :], wq_flat[:, :TCH],
                start=True, stop=True,
            )

        # b-major pipeline: batch 0 projects upfront; batch b+1's
        # projections are interleaved into batch b's attention as PE filler,
        # fragmented so no single j-iter gets a large PE burst (keeps the
        # ACT pipeline fed and the HAM clock gate warm). Each block also
        # carries the previous block's o-proj as filler.
        x_t = load_x_batch(0, split=2)
        kv_t = load_kv_batch(0, split=4)
        for fn in q_proj_frags(x_t, 0, 0) + q_proj_frags(x_t, 0, 1):
            fn()
        for quarter in range(4):
            for fn in kv_proj_frags(kv_t, 0, quarter):
                fn()
        otail = []
        for b in range(B):
            if b + 1 < B:
                nx_t = load_x_batch(b + 1)
                nkv_t = load_kv_batch(b + 1)
                frags0 = (
                    q_proj_frags(nx_t, b + 1, 0)
                    + q_proj_frags(nx_t, b + 1, 1)
                    + kv_proj_frags(nkv_t, b + 1, 0)
                )
                frags1 = (
                    kv_proj_frags(nkv_t, b + 1, 1)
                    + kv_proj_frags(nkv_t, b + 1, 2)
                    + kv_proj_frags(nkv_t, b + 1, 3)
                )
            else:
                frags0, frags1 = [], []
            # o-proj of the previous block goes late (j>=8): its aT depends
            # on the boundary normalization chain (~8us of DVE + gpsimd
            # latency), and the PE's 64-deep LDWEIGHTS pull-ahead window
            # otherwise hoists its stationary load in front of ready scores
            # work at the block boundary and stalls the in-order queue.
            f0 = list(zip((8, 10, 12, 14), otail)) + list(enumerate(frags0, start=1))
            otail = attention_block(b, 0, f0)
            f1 = list(zip((8, 10, 12, 14), otail)) + list(enumerate(frags1, start=1))
            otail = attention_block(b, 1, f1)
        for fn in otail:
            fn()

    nc.compile()
    return nc


_NC_CACHE = None


def _get_nc():
    global _NC_CACHE
    if _NC_CACHE is None:
        _NC_CACHE = build_nc()
    return _NC_CACHE


def make_in_maps(query, key_value, wq, wk, wv, wo):
    q2 = np.ascontiguousarray(np.asarray(query, np.float32).reshape(BT, D))
    kv2 = np.ascontiguousarray(np.asarray(key_value, np.float32).reshape(BS, D))
    xT = np.ascontiguousarray(q2.astype(NPBF).T)
    kvT = np.ascontiguousarray(kv2.astype(NPBF).T)
    wq = np.asarray(wq, np.float32)
    wk = np.asarray(wk, np.float32)
    wv = np.asarray(wv, np.float32)
    wo = np.asarray(wo, np.float32)
    in_maps = []
    for c in range(NCORES):
        cs = slice(c * P, (c + 1) * P)
        in_maps.append({
            "xT": xT,
            "kvT": kvT,
            "wqT": np.ascontiguousarray(wq[cs, :].astype(NPBF).T),
            "wkT": np.ascontiguousarray(wk[cs, :].astype(NPBF).T),
            "wvT": np.ascontiguousarray(wv[cs, :].astype(NPBF).T),
            "woT": np.ascontiguousarray(wo[:, cs].astype(NPBF).T),
            "eye2": np.eye(2, dtype=np.float32),
        })
    return in_maps


def run(inputs, trace=False, **kwargs):
    from concourse.bass_utils import run_bass_kernel_spmd

    nc = _get_nc()
    in_maps = make_in_maps(**inputs)
    res = run_bass_kernel_spmd(
        nc, in_maps, core_ids=list(range(NCORES)), trace=trace, **kwargs
    )
    acc = np.zeros((BT, D), np.float64)
    for r in res.results:
        acc += r["out"].astype(np.float64)
    return acc.astype(np.float32).reshape(B, T, D), res


def kernel(**inputs):
    return run(inputs, trace=False)[0]
